# revision 27
# baseline (speedup 1.0000x reference)
"""Trainium2 Bass kernel for nn_CAFF_3100966388292.

Dual-stream (SAR/OPT) cross-attention fusion net, pure data parallel
(4 samples/core on 8 cores). v2: all-fp8 datapath.

Key structure (validated numerically in sim_quant.py; the attention term is
~1e-4 of the output magnitude, so the whole attention path runs in e4m3,
while the dominant residual-colsum path uses an fp8 error-feedback pair
x ~ x8 + r8 which is *more* accurate than a bf16 colsum):
  * inputs only as e4m3 (x8) + e4m3 residual (r8); no bf16 input DMA.
  * theta/phi projections fp8 DoubleRow (as before), outputs pj e4m3.
  * logits computed TRANSPOSED (keys m on partitions) fp8 DR.
  * E = exp(L - 15) stored e4m3; S = (Ex*256)*Ey e4m3 via one
    scalar_tensor_tensor (split across DVE and GpSimd).
  * g projections fp8 DR with wbar = (ga/C)*W_w.sum(0) folded into the
    g_x weights (scaled 2^16; g_y scaled 2^6); the residual colsum rides
    as an extra weight column (2^-9 exact) over both x8 and r8 chunks.
  * softmax denominators Zx, Zy via ones-stationary fp8 DR row matmuls
    over E8; epsilon floors make fp8-dead rows yield 0 attention, not NaN.
  * apply is FLIPPED: lhsT = S chunks (stationary), rhs = gT, so U lands
    as (n-part, ci-free); a scalar_tensor_tensor with accum_out reduces
    (Ux*INV)*Uy over ci directly into a per-n column -> no row-space
    fixup, no 1-lane DVE ops, no transposes of the pooled row.
  * 1/(ZxZy)^2 computed in column space: transpose p1 row once (6 PE
    transposes), then [128,6] reciprocal/square on DVE.
  * per-sample head accumulation + per-sample output DMA -> short tail.
  * DMA spread over 3 hw queues: sync=weights, vector=x-side inputs,
    gpsimd=y-side inputs, scalar=output rows.
"""

import sys
import types

import ml_dtypes
import numpy as np

try:  # pragma: no cover
    import antenv.axon_hooks  # noqa: F401
except ImportError:
    try:
        from trn_agent_boot.trn_boot import _ntff_profile_via_ctypes

        _hook = _ntff_profile_via_ctypes("/opt/axon/libaxon_pjrt.so")
        _mod = types.ModuleType("antenv.axon_hooks")
        _mod.get_axon_ntff_profile_hook = lambda: _hook
        _mod.set_axon_ntff_profile_hook = lambda h: None
        sys.modules["antenv.axon_hooks"] = _mod
    except Exception:
        pass

import concourse.bass as bass  # noqa: F401
import concourse.tile as tile
from concourse import bacc, mybir
from concourse.alu_op_type import AluOpType
from concourse.bass_utils import run_bass_kernel_spmd

F32 = mybir.dt.float32
BF16 = mybir.dt.bfloat16
FP8 = mybir.dt.float8e4

B, C, CI, N, HOUT = 32, 512, 256, 768, 256
NCORES = 8
BPC = B // NCORES
KC = C // 128   # 4 channel chunks
MC = N // 128   # 6 position chunks
CIC = CI // 128  # 2 inner-channel chunks
NH = ((0, 512), (512, 256))  # PSUM-bank-legal free splits of N

EXP_SHIFT = -17.0
GX_SCALE = 2.0 ** 14  # on wbar-folded g_x weights
GY_SCALE = 2.0 ** 6   # on g_y weights
CS_W = 2.0 ** -9      # colsum column weight (exact in e4m3); 1/C = 2^-9
# S = Ex*Ey plain; the e^-15 exp scales cancel exactly through Z in p3
INV_SCALE = 1.0 / (GX_SCALE * GY_SCALE)
Z_EPS = 1e-6

# engine split knobs (tuned from traces)
S_ON_GPSIMD = 6      # how many of the 6 S-chunks go to gpsimd (rest DVE)
GT_ON_ACT = 6        # how many of the 12 gT casts go to ACT (rest DVE)
PJ_ON_ACT = 2        # how many of the 8 pj copies go to ACT (rest DVE)

_cached = {}


def _pack(a, pad_to=None):
    """(R, F) host array -> (128, R//128 * Fp) partition-major e4m3."""
    a = np.asarray(a, dtype=np.float32)
    r, f = a.shape
    if pad_to is not None and f < pad_to:
        a = np.concatenate([a, np.zeros((r, pad_to - f), np.float32)], axis=1)
        f = pad_to
    k = r // 128
    return np.ascontiguousarray(
        a.reshape(k, 128, f).transpose(1, 0, 2).reshape(128, k * f)
    ).astype(ml_dtypes.float8_e4m3fn)


def _build(has_hb):
    nc = bacc.Bacc("TRN2", target_bir_lowering=False, debug=False)
    AF = mybir.ActivationFunctionType
    GF = 272  # padded free width of g-weight chunks (step%16==0 for DR)

    def mm(out, lhsT, rhs, start, stop):
        nc.tensor.matmul(out, lhsT, rhs, start=start, stop=stop)

    def mmdr(out, lhsT, rhs, start, stop):
        nc.tensor.matmul(out, lhsT, rhs, start=start, stop=stop,
                         perf_mode=mybir.MatmulPerfMode.DoubleRow)

    d_x8 = nc.dram_tensor("x8", [BPC, 128, KC * N], FP8, kind="ExternalInput")
    d_y8 = nc.dram_tensor("y8", [BPC, 128, KC * N], FP8, kind="ExternalInput")
    d_rx8 = nc.dram_tensor("rx8", [BPC, 128, KC * N], FP8, kind="ExternalInput")
    d_ry8 = nc.dram_tensor("ry8", [BPC, 128, KC * N], FP8, kind="ExternalInput")
    d_w = {}
    for nm in ("wt_tx", "wt_px", "wt_ty", "wt_py"):
        d_w[nm] = nc.dram_tensor(nm, [128, KC * CI], FP8, kind="ExternalInput")
    for nm in ("wt_gx", "wt_gy"):
        d_w[nm] = nc.dram_tensor(nm, [128, KC * GF], FP8, kind="ExternalInput")
    d_w["wt_gr"] = nc.dram_tensor("wt_gr", [128, 2 * GF], FP8,
                                  kind="ExternalInput")
    d_hwT = nc.dram_tensor("hwT", [128, MC * HOUT], BF16, kind="ExternalInput")
    d_ones8 = nc.dram_tensor("ones8", [128, 32], FP8, kind="ExternalInput")
    d_ident = nc.dram_tensor("ident", [4, 4], F32, kind="ExternalInput")
    d_expb = nc.dram_tensor("expb", [128, 1], F32, kind="ExternalInput")
    if has_hb:
        d_hb = nc.dram_tensor("hb", [1, HOUT], BF16, kind="ExternalInput")
        d_onesr = nc.dram_tensor("ones_row", [1, 128], BF16,
                                 kind="ExternalInput")
    d_out = nc.dram_tensor("out", [BPC, HOUT], F32, kind="ExternalOutput")

    with tile.TileContext(nc) as tc, \
            tc.tile_pool(name="wts", bufs=1) as wts, \
            tc.tile_pool(name="inp", bufs=2) as inp, \
            tc.tile_pool(name="proj", bufs=1) as proj, \
            tc.tile_pool(name="att", bufs=1) as attp, \
            tc.tile_pool(name="rows", bufs=1) as rows, \
            tc.tile_pool(name="rtmp", bufs=4) as rtmp, \
            tc.tile_pool(name="ps", bufs=4, space="PSUM") as ps:

        # ---- DMA loads: weights on sync q (use-order); x-side inputs on
        # vector q; y-side inputs on gpsimd q ----
        w_sb = {}

        def load_w(nm, cols, eng=nc.sync):
            t = wts.tile([128, KC, cols], FP8, tag=nm, name=nm)
            eng.dma_start(t[:], d_w[nm].ap().rearrange("p (k f) -> p k f", k=KC))
            return t

        # gate the first matmul on as little data as possible
        t = wts.tile([128, KC, CI], FP8, tag="wt_tx", name="wt_tx")
        w_sb["wt_tx"] = t
        nc.sync.dma_start(t[:, 0:2, :], d_w["wt_tx"].ap()[:, :2 * CI]
                          .rearrange("p (k f) -> p k f", k=2))
        x8_0 = inp.tile([128, KC, N], FP8, tag="x8", name="x8")
        nc.scalar.dma_start(x8_0[:, 0:2, :], d_x8[0][:, :2 * N]
                            .rearrange("p (k n) -> p k n", k=2))
        y8_0 = inp.tile([128, KC, N], FP8, tag="y8", name="y8")
        nc.gpsimd.dma_start(y8_0[:, 0:2, :], d_y8[0][:, :2 * N]
                            .rearrange("p (k n) -> p k n", k=2))
        nc.sync.dma_start(t[:, 2:, :], d_w["wt_tx"].ap()[:, 2 * CI:]
                          .rearrange("p (k f) -> p k f", k=KC - 2))
        nc.scalar.dma_start(x8_0[:, 2:, :], d_x8[0][:, 2 * N:]
                            .rearrange("p (k n) -> p k n", k=2))
        nc.gpsimd.dma_start(y8_0[:, 2:, :], d_y8[0][:, 2 * N:]
                            .rearrange("p (k n) -> p k n", k=2))
        w_sb["wt_px"] = load_w("wt_px", CI)
        w_sb["wt_ty"] = load_w("wt_ty", CI)
        w_sb["wt_py"] = load_w("wt_py", CI)
        w_sb["wt_gx"] = load_w("wt_gx", GF)
        w_sb["wt_gy"] = load_w("wt_gy", GF)
        wgr = wts.tile([128, 2, GF], FP8, tag="wt_gr", name="wt_gr")
        nc.sync.dma_start(wgr[:], d_w["wt_gr"].ap()
                          .rearrange("p (k f) -> p k f", k=2))
        rx8_0 = inp.tile([128, KC, N], FP8, tag="rx8", name="rx8")
        nc.scalar.dma_start(rx8_0[:], d_rx8[0].rearrange("p (k n) -> p k n",
                                                         k=KC))
        ry8_0 = inp.tile([128, KC, N], FP8, tag="ry8", name="ry8")
        nc.gpsimd.dma_start(ry8_0[:], d_ry8[0].rearrange("p (k n) -> p k n",
                                                         k=KC))
        ones8 = wts.tile([128, 2, 16], FP8, tag="ones8", name="ones8")
        nc.sync.dma_start(ones8[:], d_ones8.ap().rearrange("p (k f) -> p k f",
                                                           k=2))
        ident = wts.tile([4, 4], F32, tag="ident", name="ident")
        nc.sync.dma_start(ident[:], d_ident.ap())
        expb = wts.tile([128, 1], F32, tag="expb", name="expb")
        nc.sync.dma_start(expb[:], d_expb.ap())
        hwT = wts.tile([128, MC, HOUT], BF16, tag="hwT", name="hwT")
        nc.sync.dma_start(hwT[:], d_hwT.ap().rearrange("p (k f) -> p k f",
                                                       k=MC))
        if has_hb:
            hb = wts.tile([1, HOUT], BF16, tag="hb", name="hb")
            nc.sync.dma_start(hb[:], d_hb.ap())
            ones_row = wts.tile([1, 128], BF16, tag="ones_row", name="ones_row")
            nc.sync.dma_start(ones_row[:], d_onesr.ap())

        def load_inputs(s):
            x8 = inp.tile([128, KC, N], FP8, tag="x8", name="x8")
            y8 = inp.tile([128, KC, N], FP8, tag="y8", name="y8")
            rx8 = inp.tile([128, KC, N], FP8, tag="rx8", name="rx8")
            ry8 = inp.tile([128, KC, N], FP8, tag="ry8", name="ry8")
            nc.scalar.dma_start(x8[:], d_x8[s].rearrange("p (k n) -> p k n",
                                                         k=KC))
            nc.gpsimd.dma_start(y8[:], d_y8[s].rearrange("p (k n) -> p k n",
                                                         k=KC))
            nc.scalar.dma_start(rx8[:], d_rx8[s].rearrange("p (k n) -> p k n",
                                                           k=KC))
            nc.gpsimd.dma_start(ry8[:], d_ry8[s].rearrange("p (k n) -> p k n",
                                                           k=KC))
            return x8, y8, rx8, ry8

        in_tiles = [(x8_0, y8_0, rx8_0, ry8_0)]
        pooledT = rows.tile([128, BPC, MC], BF16, tag="pooledT", name="pooledT")

        def const_col(tag, val):
            t = wts.tile([128, 1], F32, tag=tag, name=tag)
            nc.gpsimd.memset(t[:], val)
            return t

        c_eps = const_col("c_eps", Z_EPS)
        c_inv = const_col("c_inv", INV_SCALE)
        c_gs = const_col("c_gs", gs_f)
        c_go = const_col("c_go", go_f)

        # ---------------- per-sample phase emitters ----------------
        def emit_theta_phi(s):
            """theta/phi fp8 DR projections + pj copies for sample s."""
            x8, y8 = in_tiles[s][0], in_tiles[s][1]
            s8 = {"x": x8, "y": y8}
            pj = {}
            n_copy = 0
            for st in ("x", "y"):
                for pr in ("t", "p"):
                    w = w_sb[f"wt_{pr}{st}"]
                    dst = proj.tile([128, CIC, N], FP8, tag=f"pj_{pr}{st}",
                                    name=f"pj_{pr}{st}")
                    pj[pr + st] = dst
                    for cic in range(CIC):
                        pt = ps.tile([128, N], F32, tag="ps", name="ps")
                        for kp in range(KC // 2):
                            for o, f in NH:
                                mmdr(pt[:, o:o + f],
                                     w[:, 2 * kp:2 * kp + 2,
                                       cic * 128:(cic + 1) * 128],
                                     s8[st][:, 2 * kp:2 * kp + 2, o:o + f],
                                     kp == 0, kp == KC // 2 - 1)
                        if n_copy < PJ_ON_ACT:
                            nc.scalar.copy(dst[:, cic, :], pt[:])
                        else:
                            nc.vector.tensor_copy(dst[:, cic, :], pt[:])
                        n_copy += 1
            return pj

        def emit_logits_exp(s, pj):
            """transposed logits + exp (fp8 E) + S for sample s."""
            E = {st: attp.tile([128, MC, N], FP8, tag=f"E{st}", name=f"E{st}")
                 for st in ("x", "y")}
            S = attp.tile([128, MC, N], FP8, tag="S", name="S")
            for mc_ in range(MC):
                for st in ("x", "y"):
                    pt = ps.tile([128, N], F32, tag="ps", name="ps")
                    for o, f in NH:
                        mmdr(pt[:, o:o + f],
                             pj["p" + st][:, :, mc_ * 128:(mc_ + 1) * 128],
                             pj["t" + st][:, :, o:o + f], True, True)
                    nc.scalar.activation(E[st][:, mc_, :], pt[:], AF.Exp,
                                         bias=expb[:])
                eng = nc.gpsimd if mc_ < S_ON_GPSIMD else nc.vector
                eng.tensor_mul(S[:, mc_, :], E["x"][:, mc_, :],
                               E["y"][:, mc_, :])
            return E, S

        def emit_g(s, gsx, gscol):
            """g fp8 DR projections (+ colsum col) for sample s."""
            x8, y8, rx8, ry8 = in_tiles[s]
            r8 = {"x": rx8, "y": ry8}
            s8 = {"x": x8, "y": y8}
            gT = {}
            n_cast = 0
            for st in ("x", "y"):
                w = w_sb[f"wt_g{st}"]
                dst = proj.tile([128, MC, CI], FP8, tag=f"gT{st}",
                                name=f"gT{st}")
                gT[st] = dst
                for mc_ in range(MC):
                    pt = ps.tile([128, CI + 1], F32, tag="ps", name="psg")
                    # lhsT = input chunk (c-part, n-cols); rhs = weights
                    # (c-part, ci+colsum free).  wgr is all-zero except the
                    # colsum column, identical per chunk, so one 2-chunk tile
                    # serves every kp pair of the residual r8.
                    for kp in range(KC // 2):
                        mmdr(pt[:],
                             s8[st][:, 2 * kp:2 * kp + 2,
                                    mc_ * 128:(mc_ + 1) * 128],
                             w[:, 2 * kp:2 * kp + 2, :CI + 1],
                             kp == 0, False)
                    # r8 residual only contributes the colsum column: stream
                    # a single free column instead of all 257
                    for kp in range(KC // 2):
                        mmdr(pt[:, CI:CI + 1],
                             r8[st][:, 2 * kp:2 * kp + 2,
                                    mc_ * 128:(mc_ + 1) * 128],
                             wgr[:, :, CI:CI + 1],
                             False, kp == KC // 2 - 1)
                    if n_cast < GT_ON_ACT:
                        nc.scalar.copy(dst[:, mc_, :], pt[:, :CI])
                    else:
                        nc.vector.tensor_copy(dst[:, mc_, :], pt[:, :CI])
                    n_cast += 1
                    if st == "x":
                        nc.vector.tensor_scalar_mul(
                            gsx[:, mc_:mc_ + 1], pt[:, CI:CI + 1], c_gs[:])
                    else:
                        nc.vector.scalar_tensor_tensor(
                            gscol[:, mc_:mc_ + 1], pt[:, CI:CI + 1], c_go[:],
                            gsx[:, mc_:mc_ + 1],
                            AluOpType.mult, AluOpType.add)
            return gT

        def emit_z(s, E):
            """Z row matmuls + zx copy + p1 row; returns p1 (SBUF row)."""
            ptz = {}
            for st in ("x", "y"):
                pt = ps.tile([1, N], F32, tag="ps", name=f"psz{st}")
                ptz[st] = pt
                for j in range(MC // 2):
                    for o, f in NH:
                        mmdr(pt[:, o:o + f], ones8[:, :, 0:1],
                             E[st][:, 2 * j:2 * j + 2, o:o + f],
                             j == 0, j == MC // 2 - 1)
            zx = rtmp.tile([1, N], F32, tag="zx", name="zx", bufs=2)
            nc.scalar.activation(zx[:], ptz["x"][:], AF.Identity,
                                 bias=c_eps[:1, :])
            p1 = rtmp.tile([1, N], F32, tag="p1", name="p1", bufs=2)
            nc.vector.scalar_tensor_tensor(
                p1[:], ptz["y"][:], c_eps[:1, :], zx[:],
                AluOpType.add, AluOpType.mult)
            return p1

        def emit_p3cols(s, p1):
            """transpose p1 row -> [128, MC] cols -> 1/p1^2 (emitted after
            apply so the PE never stalls on the zx/p1 chain)."""
            pcol = ps.tile([128, MC], F32, tag="ps", name="pcol")
            for j in range(MC):
                nc.tensor.transpose(pcol[:, j:j + 1],
                                    p1[:, j * 128:(j + 1) * 128],
                                    ident[:1, :1])
            p2col = rtmp.tile([128, MC], F32, tag="p2col", name="p2col",
                              bufs=2)
            nc.vector.reciprocal(p2col[:], pcol[:])
            p3col = rtmp.tile([128, MC], F32, tag="p3col", name="p3col",
                              bufs=2)
            nc.vector.tensor_mul(p3col[:], p2col[:], p2col[:])
            return p3col

        def emit_apply(s, S, gT):
            """flipped apply: U'(n,ci) psum + stt-reduce -> qcol [128, MC]."""
            qcol = rtmp.tile([128, MC], F32, tag="qcol", name="qcol", bufs=2)
            for j in range(MC):
                ptu = {}
                for st in ("x", "y"):
                    ptu[st] = ps.tile([128, CI], F32, tag="ps",
                                      name=f"psu{st}")
                for mp in range(MC // 2):
                    lhsT = S[:, 2 * mp:2 * mp + 2, j * 128:(j + 1) * 128]
                    for st in ("x", "y"):
                        mmdr(ptu[st], lhsT, gT[st][:, 2 * mp:2 * mp + 2, :],
                             mp == 0, mp == MC // 2 - 1)
                uxb = rtmp.tile([128, CI], BF16, tag="uxb", name="uxb", bufs=2)
                nc.scalar.copy(uxb[:], ptu["x"][:])
                scr = rtmp.tile([128, CI], F32, tag="uscr", name="uscr",
                                bufs=2)
                nc.vector.scalar_tensor_tensor(
                    scr[:], uxb[:], c_inv[:], ptu["y"][:],
                    AluOpType.mult, AluOpType.mult,
                    accum_out=qcol[:, j:j + 1])
            return qcol

        def emit_pooled_head(s, qcol, p3col, gscol):
            qp = rtmp.tile([128, MC], F32, tag="qp", name="qp", bufs=2)
            nc.vector.tensor_mul(qp[:], qcol[:], p3col[:])
            nc.vector.tensor_add(pooledT[:, s, :], qp[:], gscol[:])
            pt = ps.tile([1, HOUT], F32, tag="ps", name="psh")
            for j in range(MC):
                mm(pt[:], pooledT[:, s, j:j + 1], hwT[:, j, :],
                   j == 0, (j == MC - 1) and not has_hb)
            if has_hb:
                mm(pt[:], ones_row[:, :1], hb[:], False, True)
            orow = rows.tile([1, HOUT], F32, tag=f"out_sb{s}",
                             name=f"out_sb{s}")
            nc.scalar.copy(orow[:], pt[:])
            nc.scalar.dma_start(d_out[s:s + 1, :], orow[:])

        # ---------------- software-pipelined emission ----------------
        # PE order per sample: [theta/phi_s (pre-emitted), logits_s, g_s,
        #   theta/phi_{s+1}, Z_s, apply_s, head_s] so the exp_s latency on
        #   ACT hides under g_s + theta/phi_{s+1}.
        pj_next = emit_theta_phi(0)
        for s in range(BPC):
            pj = pj_next
            gsx = rtmp.tile([128, MC], F32, tag="gsx", name="gsx", bufs=2)
            gscol = rtmp.tile([128, MC], F32, tag="gscol", name="gscol",
                              bufs=2)
            E, S = emit_logits_exp(s, pj)
            gT = emit_g(s, gsx, gscol)
            if s + 1 < BPC:
                in_tiles.append(load_inputs(s + 1))
                pj_next = emit_theta_phi(s + 1)
            p1 = emit_z(s, E)
            qcol = emit_apply(s, S, gT)
            p3col = emit_p3cols(s, p1)
            emit_pooled_head(s, qcol, p3col, gscol)

    nc.compile()
    return nc


def _prepare(inputs):
    f = lambda k: np.ascontiguousarray(np.asarray(inputs[k], dtype=np.float32))
    bf = lambda a: np.ascontiguousarray(np.asarray(a, dtype=ml_dtypes.bfloat16))
    e4m3 = ml_dtypes.float8_e4m3fn
    sar, opt = f("sar"), f("opt")
    ga = float(np.asarray(inputs["gamma_att"]).reshape(-1)[0])
    go = float(np.asarray(inputs["gamma_opt"]).reshape(-1)[0])
    gs = float(np.asarray(inputs["gamma_sar"]).reshape(-1)[0])
    W_w, W_b = f("W_w"), f("W_b")
    head_w, head_b = f("head_w"), f("head_b")

    wbar = (ga / C) * W_w.sum(axis=0)  # (CI,)
    bbar = (ga / C) * float(W_b.sum())
    hb_eff = head_b + bbar * head_w.sum(axis=1)
    # g biases folded into hb_eff would be wrong (they pass through the
    # attention+product nonlinearity); they are zero in this problem, but
    # keep correctness for small nonzero biases via the wbar-weighted
    # constant shift approximation being exact only at zero.  Assert zero.
    assert not np.any(f("g_sar_b")) and not np.any(f("g_opt_b")), \
        "nonzero g biases unsupported in v2 kernel"
    assert not np.any(f("theta_sar_b")) and not np.any(f("theta_opt_b")) \
        and not np.any(f("phi_sar_b")) and not np.any(f("phi_opt_b")), \
        "nonzero theta/phi biases unsupported in v2 kernel"

    has_hb = bool(np.any(hb_eff))
    global gs_f, go_f
    gs_f, go_f = gs, go

    key = (has_hb, gs, go)
    if key not in _cached:
        _cached[key] = _build(has_hb)
    nc = _cached[key]

    def pack_in(a):
        a = a.reshape(B, KC, 128, N).transpose(0, 2, 1, 3).reshape(B, 128,
                                                                   KC * N)
        return np.ascontiguousarray(a)

    sar_p = pack_in(sar)
    opt_p = pack_in(opt)
    x8 = sar_p.astype(e4m3)
    y8 = opt_p.astype(e4m3)
    rx8 = (sar_p - x8.astype(np.float32)).astype(e4m3)
    ry8 = (opt_p - y8.astype(np.float32)).astype(e4m3)

    GF = 272
    cs_col = np.full((C, 1), CS_W, np.float32)
    gx_w = np.concatenate(
        [GX_SCALE * (f("g_sar_w") * wbar[:, None]).T, cs_col], axis=1)
    gy_w = np.concatenate([GY_SCALE * f("g_opt_w").T, cs_col], axis=1)
    gr_w = np.concatenate(
        [np.zeros((256, CI), np.float32), np.full((256, 1), CS_W, np.float32)],
        axis=1)

    common = {
        "wt_tx": _pack(f("theta_sar_w").T),
        "wt_px": _pack(f("phi_sar_w").T),
        "wt_ty": _pack(f("theta_opt_w").T),
        "wt_py": _pack(f("phi_opt_w").T),
        "wt_gx": _pack(gx_w, pad_to=GF),
        "wt_gy": _pack(gy_w, pad_to=GF),
        "wt_gr": _pack(gr_w, pad_to=GF),
        "hwT": np.ascontiguousarray(
            _pack_bf16(head_w.T)),
        "ones8": np.ones((128, 32), e4m3),
        "ident": np.eye(4, dtype=np.float32),
        "expb": np.full((128, 1), EXP_SHIFT, np.float32),
    }
    if has_hb:
        common["hb"] = bf(hb_eff.reshape(1, HOUT))
        common["ones_row"] = np.ones((1, 128), ml_dtypes.bfloat16)

    in_maps = []
    for c in range(NCORES):
        m = dict(common)
        sl = slice(c * BPC, (c + 1) * BPC)
        m["x8"] = np.ascontiguousarray(x8[sl])
        m["y8"] = np.ascontiguousarray(y8[sl])
        m["rx8"] = np.ascontiguousarray(rx8[sl])
        m["ry8"] = np.ascontiguousarray(ry8[sl])
        in_maps.append(m)
    return nc, in_maps


def _pack_bf16(a):
    a = np.asarray(a, dtype=np.float32)
    r, fdim = a.shape
    k = r // 128
    return np.ascontiguousarray(
        a.reshape(k, 128, fdim).transpose(1, 0, 2).reshape(128, k * fdim)
    ).astype(ml_dtypes.bfloat16)


def kernel(**inputs):
    nc, in_maps = _prepare(inputs)
    res = run_bass_kernel_spmd(nc, in_maps, core_ids=list(range(NCORES)))
    return np.concatenate([res.results[c]["out"] for c in range(NCORES)],
                          axis=0)


if __name__ == "__main__":
    rng = np.random.default_rng(0)
    ins = {
        "sar": rng.standard_normal((B, C, N), dtype=np.float32),
        "opt": rng.standard_normal((B, C, N), dtype=np.float32),
    }
    for nm in ("g_sar", "g_opt", "theta_sar", "theta_opt", "phi_sar",
               "phi_opt"):
        ins[nm + "_w"] = 0.02 * rng.standard_normal((CI, C), dtype=np.float32)
        ins[nm + "_b"] = np.zeros((CI,), np.float32)
    ins["W_w"] = 0.02 * rng.standard_normal((C, CI), dtype=np.float32)
    ins["W_b"] = np.zeros((C,), np.float32)
    ins["head_w"] = 0.02 * rng.standard_normal((HOUT, N), dtype=np.float32)
    ins["head_b"] = np.zeros((HOUT,), np.float32)
    ins["gamma_sar"] = np.asarray([0.3], np.float32)
    ins["gamma_opt"] = np.asarray([1.0], np.float32)
    ins["gamma_att"] = np.asarray([1.0], np.float32)
    out = kernel(**ins)
    print(out.shape, out.dtype, np.abs(out).mean())


# revision 31
# speedup vs baseline: 1.1552x; 1.1552x over previous
"""Trainium2 Bass kernel for nn_CAFF_3100966388292.

Dual-stream (SAR/OPT) cross-attention fusion net, pure data parallel
(4 samples/core on 8 cores). v2: all-fp8 datapath.

Key structure (validated numerically in sim_quant.py; the attention term is
~1e-4 of the output magnitude, so the whole attention path runs in e4m3,
while the dominant residual-colsum path uses an fp8 error-feedback pair
x ~ x8 + r8 which is *more* accurate than a bf16 colsum):
  * inputs only as e4m3 (x8) + e4m3 residual (r8); no bf16 input DMA.
  * theta/phi projections fp8 DoubleRow (as before), outputs pj e4m3.
  * logits computed TRANSPOSED (keys m on partitions) fp8 DR.
  * E = exp(L - 15) stored e4m3; S = (Ex*256)*Ey e4m3 via one
    scalar_tensor_tensor (split across DVE and GpSimd).
  * g projections fp8 DR with wbar = (ga/C)*W_w.sum(0) folded into the
    g_x weights (scaled 2^16; g_y scaled 2^6); the residual colsum rides
    as an extra weight column (2^-9 exact) over both x8 and r8 chunks.
  * softmax denominators Zx, Zy via ones-stationary fp8 DR row matmuls
    over E8; epsilon floors make fp8-dead rows yield 0 attention, not NaN.
  * apply is FLIPPED: lhsT = S chunks (stationary), rhs = gT, so U lands
    as (n-part, ci-free); a scalar_tensor_tensor with accum_out reduces
    (Ux*INV)*Uy over ci directly into a per-n column -> no row-space
    fixup, no 1-lane DVE ops, no transposes of the pooled row.
  * 1/(ZxZy)^2 computed in column space: transpose p1 row once (6 PE
    transposes), then [128,6] reciprocal/square on DVE.
  * per-sample head accumulation + per-sample output DMA -> short tail.
  * DMA spread over 3 hw queues: sync=weights, vector=x-side inputs,
    gpsimd=y-side inputs, scalar=output rows.
"""

import sys
import types

import ml_dtypes
import numpy as np

try:  # pragma: no cover
    import antenv.axon_hooks  # noqa: F401
except ImportError:
    try:
        from trn_agent_boot.trn_boot import _ntff_profile_via_ctypes

        _hook = _ntff_profile_via_ctypes("/opt/axon/libaxon_pjrt.so")
        _mod = types.ModuleType("antenv.axon_hooks")
        _mod.get_axon_ntff_profile_hook = lambda: _hook
        _mod.set_axon_ntff_profile_hook = lambda h: None
        sys.modules["antenv.axon_hooks"] = _mod
    except Exception:
        pass

import concourse.bass as bass  # noqa: F401
import concourse.tile as tile
from concourse import bacc, mybir
from concourse.alu_op_type import AluOpType
from concourse.bass_utils import run_bass_kernel_spmd

F32 = mybir.dt.float32
BF16 = mybir.dt.bfloat16
FP8 = mybir.dt.float8e4

B, C, CI, N, HOUT = 32, 512, 256, 768, 256
NCORES = 8
BPC = B // NCORES
KC = C // 128   # 4 channel chunks
MC = N // 128   # 6 position chunks
CIC = CI // 128  # 2 inner-channel chunks
NH = ((0, 512), (512, 256))  # PSUM-bank-legal free splits of N

EXP_SHIFT = -17.0
GX_SCALE = 2.0 ** 14  # on wbar-folded g_x weights
GY_SCALE = 2.0 ** 6   # on g_y weights
CS_W = 2.0 ** -9      # colsum column weight (exact in e4m3); 1/C = 2^-9
# S = Ex*Ey plain; the e^-15 exp scales cancel exactly through Z in p3
INV_SCALE = 1.0 / (GX_SCALE * GY_SCALE)
Z_EPS = 1e-6

# engine split knobs (tuned from traces)
S_ON_GPSIMD = 3      # how many of the 6 S-chunks go to gpsimd (rest DVE)
GT_ON_ACT = 6        # how many of the 12 gT casts go to ACT (rest DVE)
PJ_ON_ACT = 4        # how many of the 8 pj copies go to ACT (rest DVE)

_cached = {}


def _pack(a, pad_to=None):
    """(R, F) host array -> (128, R//128 * Fp) partition-major e4m3."""
    a = np.asarray(a, dtype=np.float32)
    r, f = a.shape
    if pad_to is not None and f < pad_to:
        a = np.concatenate([a, np.zeros((r, pad_to - f), np.float32)], axis=1)
        f = pad_to
    k = r // 128
    return np.ascontiguousarray(
        a.reshape(k, 128, f).transpose(1, 0, 2).reshape(128, k * f)
    ).astype(ml_dtypes.float8_e4m3fn)


def _build(has_hb):
    nc = bacc.Bacc("TRN2", target_bir_lowering=False, debug=False)
    AF = mybir.ActivationFunctionType
    GF = 272  # padded free width of g-weight chunks (step%16==0 for DR)

    def mm(out, lhsT, rhs, start, stop):
        nc.tensor.matmul(out, lhsT, rhs, start=start, stop=stop)

    def mmdr(out, lhsT, rhs, start, stop):
        nc.tensor.matmul(out, lhsT, rhs, start=start, stop=stop,
                         perf_mode=mybir.MatmulPerfMode.DoubleRow)

    d_x8 = nc.dram_tensor("x8", [BPC, 128, KC * N], FP8, kind="ExternalInput")
    d_y8 = nc.dram_tensor("y8", [BPC, 128, KC * N], FP8, kind="ExternalInput")
    d_rx8 = nc.dram_tensor("rx8", [BPC, 128, KC * N], FP8, kind="ExternalInput")
    d_ry8 = nc.dram_tensor("ry8", [BPC, 128, KC * N], FP8, kind="ExternalInput")
    d_w = {}
    for nm in ("wt_tx", "wt_px", "wt_ty", "wt_py"):
        d_w[nm] = nc.dram_tensor(nm, [128, KC * CI], FP8, kind="ExternalInput")
    for nm in ("wt_gx", "wt_gy"):
        d_w[nm] = nc.dram_tensor(nm, [128, KC * GF], FP8, kind="ExternalInput")
    d_w["wt_gr"] = nc.dram_tensor("wt_gr", [128, 2 * GF], FP8,
                                  kind="ExternalInput")
    d_hwT = nc.dram_tensor("hwT", [128, MC * HOUT], BF16, kind="ExternalInput")
    d_ones8 = nc.dram_tensor("ones8", [128, 32], FP8, kind="ExternalInput")
    d_ident = nc.dram_tensor("ident", [4, 4], F32, kind="ExternalInput")
    d_expb = nc.dram_tensor("expb", [128, 1], F32, kind="ExternalInput")
    if has_hb:
        d_hb = nc.dram_tensor("hb", [1, HOUT], BF16, kind="ExternalInput")
        d_onesr = nc.dram_tensor("ones_row", [1, 128], BF16,
                                 kind="ExternalInput")
    d_out = nc.dram_tensor("out", [BPC, HOUT], F32, kind="ExternalOutput")

    with tile.TileContext(nc) as tc, \
            tc.tile_pool(name="wts", bufs=1) as wts, \
            tc.tile_pool(name="inp", bufs=2) as inp, \
            tc.tile_pool(name="proj", bufs=1) as proj, \
            tc.tile_pool(name="att", bufs=1) as attp, \
            tc.tile_pool(name="rows", bufs=1) as rows, \
            tc.tile_pool(name="rtmp", bufs=4) as rtmp, \
            tc.tile_pool(name="ps", bufs=3, space="PSUM") as ps, \
            tc.tile_pool(name="psg", bufs=2, space="PSUM") as psg:

        # ---- DMA loads: weights on sync q (use-order); x-side inputs on
        # vector q; y-side inputs on gpsimd q ----
        w_sb = {}

        def load_w(nm, cols, eng=nc.sync):
            t = wts.tile([128, KC, cols], FP8, tag=nm, name=nm)
            eng.dma_start(t[:], d_w[nm].ap().rearrange("p (k f) -> p k f", k=KC))
            return t

        # gate the first matmul on as little data as possible
        t = wts.tile([128, KC, CI], FP8, tag="wt_tx", name="wt_tx")
        w_sb["wt_tx"] = t
        nc.sync.dma_start(t[:, 0:2, :], d_w["wt_tx"].ap()[:, :2 * CI]
                          .rearrange("p (k f) -> p k f", k=2))
        x8_0 = inp.tile([128, KC, N], FP8, tag="x8", name="x8")
        nc.scalar.dma_start(x8_0[:, 0:2, :], d_x8[0][:, :2 * N]
                            .rearrange("p (k n) -> p k n", k=2))
        y8_0 = inp.tile([128, KC, N], FP8, tag="y8", name="y8")
        nc.sync.dma_start(y8_0[:, 0:2, :], d_y8[0][:, :2 * N]
                            .rearrange("p (k n) -> p k n", k=2))
        nc.sync.dma_start(t[:, 2:, :], d_w["wt_tx"].ap()[:, 2 * CI:]
                          .rearrange("p (k f) -> p k f", k=KC - 2))
        nc.scalar.dma_start(x8_0[:, 2:, :], d_x8[0][:, 2 * N:]
                            .rearrange("p (k n) -> p k n", k=2))
        nc.sync.dma_start(y8_0[:, 2:, :], d_y8[0][:, 2 * N:]
                            .rearrange("p (k n) -> p k n", k=2))
        w_sb["wt_px"] = load_w("wt_px", CI)
        w_sb["wt_ty"] = load_w("wt_ty", CI)
        w_sb["wt_py"] = load_w("wt_py", CI)
        w_sb["wt_gx"] = load_w("wt_gx", GF)
        w_sb["wt_gy"] = load_w("wt_gy", GF)
        wgr = wts.tile([128, 2, GF], FP8, tag="wt_gr", name="wt_gr")
        nc.sync.dma_start(wgr[:], d_w["wt_gr"].ap()
                          .rearrange("p (k f) -> p k f", k=2))
        rx8_0 = inp.tile([128, KC, N], FP8, tag="rx8", name="rx8")
        nc.scalar.dma_start(rx8_0[:], d_rx8[0].rearrange("p (k n) -> p k n",
                                                         k=KC))
        ry8_0 = inp.tile([128, KC, N], FP8, tag="ry8", name="ry8")
        nc.sync.dma_start(ry8_0[:], d_ry8[0].rearrange("p (k n) -> p k n",
                                                         k=KC))
        ones8 = wts.tile([128, 2, 16], FP8, tag="ones8", name="ones8")
        nc.sync.dma_start(ones8[:], d_ones8.ap().rearrange("p (k f) -> p k f",
                                                           k=2))
        ident = wts.tile([4, 4], F32, tag="ident", name="ident")
        nc.sync.dma_start(ident[:], d_ident.ap())
        expb = wts.tile([128, 1], F32, tag="expb", name="expb")
        nc.sync.dma_start(expb[:], d_expb.ap())
        hwT = wts.tile([128, MC, HOUT], BF16, tag="hwT", name="hwT")
        nc.sync.dma_start(hwT[:], d_hwT.ap().rearrange("p (k f) -> p k f",
                                                       k=MC))
        if has_hb:
            hb = wts.tile([1, HOUT], BF16, tag="hb", name="hb")
            nc.sync.dma_start(hb[:], d_hb.ap())
            ones_row = wts.tile([1, 128], BF16, tag="ones_row", name="ones_row")
            nc.sync.dma_start(ones_row[:], d_onesr.ap())

        def load_inputs(s):
            x8 = inp.tile([128, KC, N], FP8, tag="x8", name="x8")
            y8 = inp.tile([128, KC, N], FP8, tag="y8", name="y8")
            rx8 = inp.tile([128, KC, N], FP8, tag="rx8", name="rx8")
            ry8 = inp.tile([128, KC, N], FP8, tag="ry8", name="ry8")
            nc.scalar.dma_start(x8[:], d_x8[s].rearrange("p (k n) -> p k n",
                                                         k=KC))
            nc.sync.dma_start(y8[:], d_y8[s].rearrange("p (k n) -> p k n",
                                                         k=KC))
            nc.scalar.dma_start(rx8[:], d_rx8[s].rearrange("p (k n) -> p k n",
                                                           k=KC))
            nc.sync.dma_start(ry8[:], d_ry8[s].rearrange("p (k n) -> p k n",
                                                           k=KC))
            return x8, y8, rx8, ry8

        in_tiles = [(x8_0, y8_0, rx8_0, ry8_0)]
        pooledT = rows.tile([128, BPC, MC], BF16, tag="pooledT", name="pooledT")

        def const_col(tag, val):
            t = wts.tile([128, 1], F32, tag=tag, name=tag)
            nc.gpsimd.memset(t[:], val)
            return t

        c_eps = const_col("c_eps", Z_EPS)
        c_inv = const_col("c_inv", INV_SCALE)
        c_gs = const_col("c_gs", gs_f)
        c_go = const_col("c_go", go_f)

        # ---------------- per-sample phase emitters ----------------
        def emit_theta_phi(s):
            """theta/phi fp8 DR projections + pj copies for sample s."""
            x8, y8 = in_tiles[s][0], in_tiles[s][1]
            s8 = {"x": x8, "y": y8}
            pj = {}
            n_copy = 0
            for st in ("x", "y"):
                for pr in ("t", "p"):
                    w = w_sb[f"wt_{pr}{st}"]
                    dst = proj.tile([128, CIC, N], FP8, tag=f"pj_{pr}{st}",
                                    name=f"pj_{pr}{st}")
                    pj[pr + st] = dst
                    for cic in range(CIC):
                        pt = ps.tile([128, N], F32, tag="ps", name="ps")
                        for kp in range(KC // 2):
                            for o, f in NH:
                                mmdr(pt[:, o:o + f],
                                     w[:, 2 * kp:2 * kp + 2,
                                       cic * 128:(cic + 1) * 128],
                                     s8[st][:, 2 * kp:2 * kp + 2, o:o + f],
                                     kp == 0, kp == KC // 2 - 1)
                        if n_copy < PJ_ON_ACT:
                            nc.scalar.copy(dst[:, cic, :], pt[:])
                        else:
                            nc.vector.tensor_copy(dst[:, cic, :], pt[:])
                        n_copy += 1
            return pj

        def emit_mid(s, pj, gsx, gscol):
            """Interleaved logits+exp+S and g projections, per mc chunk.

            The logits psums (ps pool) are freed by ACT exp; the g psums
            (psg pool) are freed by the gT casts (ACT/DVE); interleaving
            keeps the PE busy on whichever pool has a free slot instead of
            serially stalling on one consumer.
            """
            x8, y8, rx8, ry8 = in_tiles[s]
            r8 = {"x": rx8, "y": ry8}
            s8 = {"x": x8, "y": y8}
            E = {st: attp.tile([128, MC, N], FP8, tag=f"E{st}", name=f"E{st}")
                 for st in ("x", "y")}
            S = attp.tile([128, MC, N], FP8, tag="S", name="S")
            gT = {st: proj.tile([128, MC, CI], FP8, tag=f"gT{st}",
                                name=f"gT{st}") for st in ("x", "y")}
            for mc_ in range(MC):
                for st in ("x", "y"):
                    pt = ps.tile([128, N], F32, tag="ps", name="ps")
                    for o, f in NH:
                        mmdr(pt[:, o:o + f],
                             pj["p" + st][:, :, mc_ * 128:(mc_ + 1) * 128],
                             pj["t" + st][:, :, o:o + f], True, True)
                    nc.scalar.activation(E[st][:, mc_, :], pt[:], AF.Exp,
                                         bias=expb[:])
                eng = nc.gpsimd if mc_ < S_ON_GPSIMD else nc.vector
                eng.tensor_mul(S[:, mc_, :], E["x"][:, mc_, :],
                               E["y"][:, mc_, :])
                for st in ("x", "y"):
                    w = w_sb[f"wt_g{st}"]
                    pt = psg.tile([128, CI + 1], F32, tag="psg", name="psg")
                    # lhsT = input chunk (c-part, n-cols); rhs = weights
                    # (c-part, ci+colsum free).  wgr is all-zero except the
                    # colsum column (identical per chunk), and the r8
                    # residual only contributes that column: stream 1 col.
                    for kp in range(KC // 2):
                        mmdr(pt[:],
                             s8[st][:, 2 * kp:2 * kp + 2,
                                    mc_ * 128:(mc_ + 1) * 128],
                             w[:, 2 * kp:2 * kp + 2, :CI + 1],
                             kp == 0, False)
                    for kp in range(KC // 2):
                        mmdr(pt[:, CI:CI + 1],
                             r8[st][:, 2 * kp:2 * kp + 2,
                                    mc_ * 128:(mc_ + 1) * 128],
                             wgr[:, :, CI:CI + 1],
                             False, kp == KC // 2 - 1)
                    if st == "x":
                        nc.scalar.copy(gT[st][:, mc_, :], pt[:, :CI])
                        nc.vector.tensor_scalar_mul(
                            gsx[:, mc_:mc_ + 1], pt[:, CI:CI + 1], c_gs[:])
                    else:
                        nc.vector.tensor_copy(gT[st][:, mc_, :], pt[:, :CI])
                        nc.vector.scalar_tensor_tensor(
                            gscol[:, mc_:mc_ + 1], pt[:, CI:CI + 1], c_go[:],
                            gsx[:, mc_:mc_ + 1],
                            AluOpType.mult, AluOpType.add)
            return E, S, gT

        def emit_z(s, E):
            """Z row matmuls + zx copy + p1 row; returns p1 (SBUF row)."""
            ptz = {}
            for st in ("x", "y"):
                pt = ps.tile([1, N], F32, tag="ps", name=f"psz{st}")
                ptz[st] = pt
                for j in range(MC // 2):
                    for o, f in NH:
                        mmdr(pt[:, o:o + f], ones8[:, :, 0:1],
                             E[st][:, 2 * j:2 * j + 2, o:o + f],
                             j == 0, j == MC // 2 - 1)
            zx = rtmp.tile([1, N], F32, tag="zx", name="zx", bufs=2)
            nc.scalar.activation(zx[:], ptz["x"][:], AF.Identity,
                                 bias=c_eps[:1, :])
            p1 = rtmp.tile([1, N], F32, tag="p1", name="p1", bufs=2)
            nc.vector.scalar_tensor_tensor(
                p1[:], ptz["y"][:], c_eps[:1, :], zx[:],
                AluOpType.add, AluOpType.mult)
            return p1

        def emit_p3cols(s, p1):
            """transpose p1 row -> [128, MC] cols -> 1/p1^2 (emitted after
            apply so the PE never stalls on the zx/p1 chain)."""
            pcol = psg.tile([128, MC], F32, tag="psg", name="pcol")
            for j in range(MC):
                nc.tensor.transpose(pcol[:, j:j + 1],
                                    p1[:, j * 128:(j + 1) * 128],
                                    ident[:1, :1])
            p2col = rtmp.tile([128, MC], F32, tag="p2col", name="p2col",
                              bufs=2)
            nc.vector.reciprocal(p2col[:], pcol[:])
            p3col = rtmp.tile([128, MC], F32, tag="p3col", name="p3col",
                              bufs=2)
            nc.vector.tensor_mul(p3col[:], p2col[:], p2col[:])
            return p3col

        def emit_apply(s, S, gT):
            """flipped apply: U'(n,ci) psum + stt-reduce -> qcol [128, MC]."""
            qcol = rtmp.tile([128, MC], F32, tag="qcol", name="qcol", bufs=2)
            for j in range(MC):
                ptu = {}
                for st in ("x", "y"):
                    ptu[st] = ps.tile([128, CI], F32, tag="ps",
                                      name=f"psu{st}")
                for mp in range(MC // 2):
                    lhsT = S[:, 2 * mp:2 * mp + 2, j * 128:(j + 1) * 128]
                    for st in ("x", "y"):
                        mmdr(ptu[st], lhsT, gT[st][:, 2 * mp:2 * mp + 2, :],
                             mp == 0, mp == MC // 2 - 1)
                uxb = rtmp.tile([128, CI], BF16, tag="uxb", name="uxb", bufs=2)
                nc.scalar.copy(uxb[:], ptu["x"][:])
                scr = rtmp.tile([128, CI], F32, tag="uscr", name="uscr",
                                bufs=2)
                nc.vector.scalar_tensor_tensor(
                    scr[:], uxb[:], c_inv[:], ptu["y"][:],
                    AluOpType.mult, AluOpType.mult,
                    accum_out=qcol[:, j:j + 1])
            return qcol

        def emit_pooled_head(s, qcol, p3col, gscol):
            qp = rtmp.tile([128, MC], F32, tag="qp", name="qp", bufs=2)
            nc.vector.tensor_mul(qp[:], qcol[:], p3col[:])
            nc.vector.tensor_add(pooledT[:, s, :], qp[:], gscol[:])
            pt = psg.tile([1, HOUT], F32, tag="psg", name="psh")
            for j in range(MC):
                mm(pt[:], pooledT[:, s, j:j + 1], hwT[:, j, :],
                   j == 0, (j == MC - 1) and not has_hb)
            if has_hb:
                mm(pt[:], ones_row[:, :1], hb[:], False, True)
            orow = rows.tile([1, HOUT], F32, tag=f"out_sb{s}",
                             name=f"out_sb{s}")
            nc.scalar.copy(orow[:], pt[:])
            nc.scalar.dma_start(d_out[s:s + 1, :], orow[:])

        # ---------------- software-pipelined emission ----------------
        # PE order per sample: [theta/phi_s (pre-emitted), logits_s, g_s,
        #   theta/phi_{s+1}, Z_s, apply_s, head_s] so the exp_s latency on
        #   ACT hides under g_s + theta/phi_{s+1}.
        pj_next = emit_theta_phi(0)
        pending = None
        for s in range(BPC):
            pj = pj_next
            gsx = rtmp.tile([128, MC], F32, tag="gsx", name="gsx", bufs=2)
            gscol = rtmp.tile([128, MC], F32, tag="gscol", name="gscol",
                              bufs=2)
            if s + 1 < BPC:
                in_tiles.append(load_inputs(s + 1))
            E, S, gT = emit_mid(s, pj, gsx, gscol)
            if pending is not None:
                emit_pooled_head(*pending)
            if s + 1 < BPC:
                pj_next = emit_theta_phi(s + 1)
            p1 = emit_z(s, E)
            qcol = emit_apply(s, S, gT)
            p3col = emit_p3cols(s, p1)
            pending = (s, qcol, p3col, gscol)
        emit_pooled_head(*pending)

    nc.compile()
    return nc


def _prepare(inputs):
    f = lambda k: np.ascontiguousarray(np.asarray(inputs[k], dtype=np.float32))
    bf = lambda a: np.ascontiguousarray(np.asarray(a, dtype=ml_dtypes.bfloat16))
    e4m3 = ml_dtypes.float8_e4m3fn
    sar, opt = f("sar"), f("opt")
    ga = float(np.asarray(inputs["gamma_att"]).reshape(-1)[0])
    go = float(np.asarray(inputs["gamma_opt"]).reshape(-1)[0])
    gs = float(np.asarray(inputs["gamma_sar"]).reshape(-1)[0])
    W_w, W_b = f("W_w"), f("W_b")
    head_w, head_b = f("head_w"), f("head_b")

    wbar = (ga / C) * W_w.sum(axis=0)  # (CI,)
    bbar = (ga / C) * float(W_b.sum())
    hb_eff = head_b + bbar * head_w.sum(axis=1)
    # g biases folded into hb_eff would be wrong (they pass through the
    # attention+product nonlinearity); they are zero in this problem, but
    # keep correctness for small nonzero biases via the wbar-weighted
    # constant shift approximation being exact only at zero.  Assert zero.
    assert not np.any(f("g_sar_b")) and not np.any(f("g_opt_b")), \
        "nonzero g biases unsupported in v2 kernel"
    assert not np.any(f("theta_sar_b")) and not np.any(f("theta_opt_b")) \
        and not np.any(f("phi_sar_b")) and not np.any(f("phi_opt_b")), \
        "nonzero theta/phi biases unsupported in v2 kernel"

    has_hb = bool(np.any(hb_eff))
    global gs_f, go_f
    gs_f, go_f = gs, go

    key = (has_hb, gs, go)
    if key not in _cached:
        _cached[key] = _build(has_hb)
    nc = _cached[key]

    def pack_in(a):
        a = a.reshape(B, KC, 128, N).transpose(0, 2, 1, 3).reshape(B, 128,
                                                                   KC * N)
        return np.ascontiguousarray(a)

    sar_p = pack_in(sar)
    opt_p = pack_in(opt)
    x8 = sar_p.astype(e4m3)
    y8 = opt_p.astype(e4m3)
    rx8 = (sar_p - x8.astype(np.float32)).astype(e4m3)
    ry8 = (opt_p - y8.astype(np.float32)).astype(e4m3)

    GF = 272
    cs_col = np.full((C, 1), CS_W, np.float32)
    gx_w = np.concatenate(
        [GX_SCALE * (f("g_sar_w") * wbar[:, None]).T, cs_col], axis=1)
    gy_w = np.concatenate([GY_SCALE * f("g_opt_w").T, cs_col], axis=1)
    gr_w = np.concatenate(
        [np.zeros((256, CI), np.float32), np.full((256, 1), CS_W, np.float32)],
        axis=1)

    common = {
        "wt_tx": _pack(f("theta_sar_w").T),
        "wt_px": _pack(f("phi_sar_w").T),
        "wt_ty": _pack(f("theta_opt_w").T),
        "wt_py": _pack(f("phi_opt_w").T),
        "wt_gx": _pack(gx_w, pad_to=GF),
        "wt_gy": _pack(gy_w, pad_to=GF),
        "wt_gr": _pack(gr_w, pad_to=GF),
        "hwT": np.ascontiguousarray(
            _pack_bf16(head_w.T)),
        "ones8": np.ones((128, 32), e4m3),
        "ident": np.eye(4, dtype=np.float32),
        "expb": np.full((128, 1), EXP_SHIFT, np.float32),
    }
    if has_hb:
        common["hb"] = bf(hb_eff.reshape(1, HOUT))
        common["ones_row"] = np.ones((1, 128), ml_dtypes.bfloat16)

    in_maps = []
    for c in range(NCORES):
        m = dict(common)
        sl = slice(c * BPC, (c + 1) * BPC)
        m["x8"] = np.ascontiguousarray(x8[sl])
        m["y8"] = np.ascontiguousarray(y8[sl])
        m["rx8"] = np.ascontiguousarray(rx8[sl])
        m["ry8"] = np.ascontiguousarray(ry8[sl])
        in_maps.append(m)
    return nc, in_maps


def _pack_bf16(a):
    a = np.asarray(a, dtype=np.float32)
    r, fdim = a.shape
    k = r // 128
    return np.ascontiguousarray(
        a.reshape(k, 128, fdim).transpose(1, 0, 2).reshape(128, k * fdim)
    ).astype(ml_dtypes.bfloat16)


def kernel(**inputs):
    nc, in_maps = _prepare(inputs)
    res = run_bass_kernel_spmd(nc, in_maps, core_ids=list(range(NCORES)))
    return np.concatenate([res.results[c]["out"] for c in range(NCORES)],
                          axis=0)


if __name__ == "__main__":
    rng = np.random.default_rng(0)
    ins = {
        "sar": rng.standard_normal((B, C, N), dtype=np.float32),
        "opt": rng.standard_normal((B, C, N), dtype=np.float32),
    }
    for nm in ("g_sar", "g_opt", "theta_sar", "theta_opt", "phi_sar",
               "phi_opt"):
        ins[nm + "_w"] = 0.02 * rng.standard_normal((CI, C), dtype=np.float32)
        ins[nm + "_b"] = np.zeros((CI,), np.float32)
    ins["W_w"] = 0.02 * rng.standard_normal((C, CI), dtype=np.float32)
    ins["W_b"] = np.zeros((C,), np.float32)
    ins["head_w"] = 0.02 * rng.standard_normal((HOUT, N), dtype=np.float32)
    ins["head_b"] = np.zeros((HOUT,), np.float32)
    ins["gamma_sar"] = np.asarray([0.3], np.float32)
    ins["gamma_opt"] = np.asarray([1.0], np.float32)
    ins["gamma_att"] = np.asarray([1.0], np.float32)
    out = kernel(**ins)
    print(out.shape, out.dtype, np.abs(out).mean())


# revision 33
# speedup vs baseline: 1.2136x; 1.0505x over previous
"""Trainium2 Bass kernel for nn_CAFF_3100966388292.

Dual-stream (SAR/OPT) cross-attention fusion net, pure data parallel
(4 samples/core on 8 cores). v2: all-fp8 datapath.

Key structure (validated numerically in sim_quant.py; the attention term is
~1e-4 of the output magnitude, so the whole attention path runs in e4m3,
while the dominant residual-colsum path uses an fp8 error-feedback pair
x ~ x8 + r8 which is *more* accurate than a bf16 colsum):
  * inputs only as e4m3 (x8) + e4m3 residual (r8); no bf16 input DMA.
  * theta/phi projections fp8 DoubleRow (as before), outputs pj e4m3.
  * logits computed TRANSPOSED (keys m on partitions) fp8 DR.
  * E = exp(L - 15) stored e4m3; S = (Ex*256)*Ey e4m3 via one
    scalar_tensor_tensor (split across DVE and GpSimd).
  * g projections fp8 DR with wbar = (ga/C)*W_w.sum(0) folded into the
    g_x weights (scaled 2^16; g_y scaled 2^6); the residual colsum rides
    as an extra weight column (2^-9 exact) over both x8 and r8 chunks.
  * softmax denominators Zx, Zy via ones-stationary fp8 DR row matmuls
    over E8; epsilon floors make fp8-dead rows yield 0 attention, not NaN.
  * apply is FLIPPED: lhsT = S chunks (stationary), rhs = gT, so U lands
    as (n-part, ci-free); a scalar_tensor_tensor with accum_out reduces
    (Ux*INV)*Uy over ci directly into a per-n column -> no row-space
    fixup, no 1-lane DVE ops, no transposes of the pooled row.
  * 1/(ZxZy)^2 computed in column space: transpose p1 row once (6 PE
    transposes), then [128,6] reciprocal/square on DVE.
  * per-sample head accumulation + per-sample output DMA -> short tail.
  * DMA spread over 3 hw queues: sync=weights, vector=x-side inputs,
    gpsimd=y-side inputs, scalar=output rows.
"""

import sys
import types

import ml_dtypes
import numpy as np

try:  # pragma: no cover
    import antenv.axon_hooks  # noqa: F401
except ImportError:
    try:
        from trn_agent_boot.trn_boot import _ntff_profile_via_ctypes

        _hook = _ntff_profile_via_ctypes("/opt/axon/libaxon_pjrt.so")
        _mod = types.ModuleType("antenv.axon_hooks")
        _mod.get_axon_ntff_profile_hook = lambda: _hook
        _mod.set_axon_ntff_profile_hook = lambda h: None
        sys.modules["antenv.axon_hooks"] = _mod
    except Exception:
        pass

import concourse.bass as bass  # noqa: F401
import concourse.tile as tile
from concourse import bacc, mybir
from concourse.alu_op_type import AluOpType
from concourse.bass_utils import run_bass_kernel_spmd

F32 = mybir.dt.float32
BF16 = mybir.dt.bfloat16
FP8 = mybir.dt.float8e4

B, C, CI, N, HOUT = 32, 512, 256, 768, 256
NCORES = 8
BPC = B // NCORES
KC = C // 128   # 4 channel chunks
MC = N // 128   # 6 position chunks
CIC = CI // 128  # 2 inner-channel chunks
NH = ((0, 512), (512, 256))  # PSUM-bank-legal free splits of N

EXP_SHIFT = -17.0
GX_SCALE = 2.0 ** 14  # on wbar-folded g_x weights
GY_SCALE = 2.0 ** 6   # on g_y weights
CS_W = 2.0 ** -9      # colsum column weight (exact in e4m3); 1/C = 2^-9
# S = Ex*Ey plain; the e^-15 exp scales cancel exactly through Z in p3
INV_SCALE = 1.0 / (GX_SCALE * GY_SCALE)
Z_EPS = 1e-6

# engine split knobs (tuned from traces)
S_ON_GPSIMD = 4      # how many of the 6 S-chunks go to gpsimd (rest DVE)
GT_ON_ACT = 2        # how many of the 12 gT casts go to ACT (rest DVE)
PJ_ON_ACT = 2        # how many of the 8 pj copies go to ACT (rest DVE)

_cached = {}


def _pack(a, pad_to=None):
    """(R, F) host array -> (128, R//128 * Fp) partition-major e4m3."""
    a = np.asarray(a, dtype=np.float32)
    r, f = a.shape
    if pad_to is not None and f < pad_to:
        a = np.concatenate([a, np.zeros((r, pad_to - f), np.float32)], axis=1)
        f = pad_to
    k = r // 128
    return np.ascontiguousarray(
        a.reshape(k, 128, f).transpose(1, 0, 2).reshape(128, k * f)
    ).astype(ml_dtypes.float8_e4m3fn)


def _build(has_hb):
    nc = bacc.Bacc("TRN2", target_bir_lowering=False, debug=False)
    AF = mybir.ActivationFunctionType
    GF = 272  # padded free width of g-weight chunks (step%16==0 for DR)

    def mm(out, lhsT, rhs, start, stop):
        nc.tensor.matmul(out, lhsT, rhs, start=start, stop=stop)

    def mmdr(out, lhsT, rhs, start, stop):
        nc.tensor.matmul(out, lhsT, rhs, start=start, stop=stop,
                         perf_mode=mybir.MatmulPerfMode.DoubleRow)

    d_x8 = nc.dram_tensor("x8", [BPC, 128, KC * N], FP8, kind="ExternalInput")
    d_y8 = nc.dram_tensor("y8", [BPC, 128, KC * N], FP8, kind="ExternalInput")
    d_rx8 = nc.dram_tensor("rx8", [BPC, 128, KC * N], FP8, kind="ExternalInput")
    d_ry8 = nc.dram_tensor("ry8", [BPC, 128, KC * N], FP8, kind="ExternalInput")
    d_w = {}
    for nm in ("wt_tx", "wt_px", "wt_ty", "wt_py"):
        d_w[nm] = nc.dram_tensor(nm, [128, KC * CI], FP8, kind="ExternalInput")
    for nm in ("wt_gx", "wt_gy"):
        d_w[nm] = nc.dram_tensor(nm, [128, KC * GF], FP8, kind="ExternalInput")
    d_w["wt_gr"] = nc.dram_tensor("wt_gr", [128, 2 * GF], FP8,
                                  kind="ExternalInput")
    d_hwT = nc.dram_tensor("hwT", [128, MC * HOUT], BF16, kind="ExternalInput")
    d_ones8 = nc.dram_tensor("ones8", [128, 32], FP8, kind="ExternalInput")
    d_ident = nc.dram_tensor("ident", [4, 4], F32, kind="ExternalInput")
    d_expb = nc.dram_tensor("expb", [128, 1], F32, kind="ExternalInput")
    if has_hb:
        d_hb = nc.dram_tensor("hb", [1, HOUT], BF16, kind="ExternalInput")
        d_onesr = nc.dram_tensor("ones_row", [1, 128], BF16,
                                 kind="ExternalInput")
    d_out = nc.dram_tensor("out", [BPC, HOUT], F32, kind="ExternalOutput")

    with tile.TileContext(nc) as tc, \
            tc.tile_pool(name="wts", bufs=1) as wts, \
            tc.tile_pool(name="inp", bufs=2) as inp, \
            tc.tile_pool(name="proj", bufs=1) as proj, \
            tc.tile_pool(name="att", bufs=1) as attp, \
            tc.tile_pool(name="rows", bufs=1) as rows, \
            tc.tile_pool(name="rtmp", bufs=4) as rtmp, \
            tc.tile_pool(name="ps", bufs=3, space="PSUM") as ps, \
            tc.tile_pool(name="psg", bufs=2, space="PSUM") as psg:

        # ---- DMA loads: weights on sync q (use-order); x-side inputs on
        # vector q; y-side inputs on gpsimd q ----
        w_sb = {}

        def load_w(nm, cols, eng=nc.sync):
            t = wts.tile([128, KC, cols], FP8, tag=nm, name=nm)
            eng.dma_start(t[:], d_w[nm].ap().rearrange("p (k f) -> p k f", k=KC))
            return t

        # gate the first matmul on as little data as possible
        t = wts.tile([128, KC, CI], FP8, tag="wt_tx", name="wt_tx")
        w_sb["wt_tx"] = t
        nc.sync.dma_start(t[:, 0:2, :], d_w["wt_tx"].ap()[:, :2 * CI]
                          .rearrange("p (k f) -> p k f", k=2))
        x8_0 = inp.tile([128, KC, N], FP8, tag="x8", name="x8")
        nc.scalar.dma_start(x8_0[:, 0:2, :], d_x8[0][:, :2 * N]
                            .rearrange("p (k n) -> p k n", k=2))
        y8_0 = inp.tile([128, KC, N], FP8, tag="y8", name="y8")
        nc.sync.dma_start(y8_0[:, 0:2, :], d_y8[0][:, :2 * N]
                            .rearrange("p (k n) -> p k n", k=2))
        nc.sync.dma_start(t[:, 2:, :], d_w["wt_tx"].ap()[:, 2 * CI:]
                          .rearrange("p (k f) -> p k f", k=KC - 2))
        nc.scalar.dma_start(x8_0[:, 2:, :], d_x8[0][:, 2 * N:]
                            .rearrange("p (k n) -> p k n", k=2))
        nc.sync.dma_start(y8_0[:, 2:, :], d_y8[0][:, 2 * N:]
                            .rearrange("p (k n) -> p k n", k=2))
        w_sb["wt_px"] = load_w("wt_px", CI)
        w_sb["wt_ty"] = load_w("wt_ty", CI)
        w_sb["wt_py"] = load_w("wt_py", CI)
        w_sb["wt_gx"] = load_w("wt_gx", GF)
        w_sb["wt_gy"] = load_w("wt_gy", GF)
        wgr = wts.tile([128, 2, GF], FP8, tag="wt_gr", name="wt_gr")
        nc.sync.dma_start(wgr[:], d_w["wt_gr"].ap()
                          .rearrange("p (k f) -> p k f", k=2))
        rx8_0 = inp.tile([128, KC, N], FP8, tag="rx8", name="rx8")
        nc.scalar.dma_start(rx8_0[:], d_rx8[0].rearrange("p (k n) -> p k n",
                                                         k=KC))
        ry8_0 = inp.tile([128, KC, N], FP8, tag="ry8", name="ry8")
        nc.sync.dma_start(ry8_0[:], d_ry8[0].rearrange("p (k n) -> p k n",
                                                         k=KC))
        ones8 = wts.tile([128, 2, 16], FP8, tag="ones8", name="ones8")
        nc.sync.dma_start(ones8[:], d_ones8.ap().rearrange("p (k f) -> p k f",
                                                           k=2))
        ident = wts.tile([4, 4], F32, tag="ident", name="ident")
        nc.sync.dma_start(ident[:], d_ident.ap())
        expb = wts.tile([128, 1], F32, tag="expb", name="expb")
        nc.sync.dma_start(expb[:], d_expb.ap())
        hwT = wts.tile([128, MC, HOUT], BF16, tag="hwT", name="hwT")
        nc.sync.dma_start(hwT[:], d_hwT.ap().rearrange("p (k f) -> p k f",
                                                       k=MC))
        if has_hb:
            hb = wts.tile([1, HOUT], BF16, tag="hb", name="hb")
            nc.sync.dma_start(hb[:], d_hb.ap())
            ones_row = wts.tile([1, 128], BF16, tag="ones_row", name="ones_row")
            nc.sync.dma_start(ones_row[:], d_onesr.ap())

        def load_inputs(s):
            x8 = inp.tile([128, KC, N], FP8, tag="x8", name="x8")
            y8 = inp.tile([128, KC, N], FP8, tag="y8", name="y8")
            rx8 = inp.tile([128, KC, N], FP8, tag="rx8", name="rx8")
            ry8 = inp.tile([128, KC, N], FP8, tag="ry8", name="ry8")
            nc.scalar.dma_start(x8[:], d_x8[s].rearrange("p (k n) -> p k n",
                                                         k=KC))
            nc.sync.dma_start(y8[:], d_y8[s].rearrange("p (k n) -> p k n",
                                                         k=KC))
            nc.scalar.dma_start(rx8[:], d_rx8[s].rearrange("p (k n) -> p k n",
                                                           k=KC))
            nc.sync.dma_start(ry8[:], d_ry8[s].rearrange("p (k n) -> p k n",
                                                           k=KC))
            return x8, y8, rx8, ry8

        in_tiles = [(x8_0, y8_0, rx8_0, ry8_0)]
        pooledT = rows.tile([128, BPC, MC], BF16, tag="pooledT", name="pooledT")

        def const_col(tag, val):
            t = wts.tile([128, 1], F32, tag=tag, name=tag)
            nc.gpsimd.memset(t[:], val)
            return t

        c_eps = const_col("c_eps", Z_EPS)
        c_inv = const_col("c_inv", INV_SCALE)
        c_gs = const_col("c_gs", gs_f)
        c_go = const_col("c_go", go_f)

        # ---------------- per-sample phase emitters ----------------
        def emit_theta_phi(s):
            """theta/phi fp8 DR projections + pj copies for sample s."""
            x8, y8 = in_tiles[s][0], in_tiles[s][1]
            s8 = {"x": x8, "y": y8}
            pj = {}
            n_copy = 0
            for st in ("x", "y"):
                for pr in ("t", "p"):
                    w = w_sb[f"wt_{pr}{st}"]
                    dst = proj.tile([128, CIC, N], FP8, tag=f"pj_{pr}{st}",
                                    name=f"pj_{pr}{st}")
                    pj[pr + st] = dst
                    for cic in range(CIC):
                        pt = ps.tile([128, N], F32, tag="ps", name="ps")
                        for kp in range(KC // 2):
                            for o, f in NH:
                                mmdr(pt[:, o:o + f],
                                     w[:, 2 * kp:2 * kp + 2,
                                       cic * 128:(cic + 1) * 128],
                                     s8[st][:, 2 * kp:2 * kp + 2, o:o + f],
                                     kp == 0, kp == KC // 2 - 1)
                        if n_copy < PJ_ON_ACT:
                            nc.scalar.copy(dst[:, cic, :], pt[:])
                        else:
                            nc.vector.tensor_copy(dst[:, cic, :], pt[:])
                        n_copy += 1
            return pj

        def emit_mid(s, pj, gsx, gscol):
            """Interleaved logits+exp+S and g projections, per mc chunk.

            The logits psums (ps pool) are freed by ACT exp; the g psums
            (psg pool) are freed by the gT casts (ACT/DVE); interleaving
            keeps the PE busy on whichever pool has a free slot instead of
            serially stalling on one consumer.
            """
            x8, y8, rx8, ry8 = in_tiles[s]
            r8 = {"x": rx8, "y": ry8}
            s8 = {"x": x8, "y": y8}
            E = {st: attp.tile([128, MC, N], FP8, tag=f"E{st}", name=f"E{st}")
                 for st in ("x", "y")}
            S = attp.tile([128, MC, N], FP8, tag="S", name="S")
            gT = {st: proj.tile([128, MC, CI], FP8, tag=f"gT{st}",
                                name=f"gT{st}") for st in ("x", "y")}
            for mc_ in range(MC):
                for st in ("x", "y"):
                    pt = ps.tile([128, N], F32, tag="ps", name="ps")
                    for o, f in NH:
                        mmdr(pt[:, o:o + f],
                             pj["p" + st][:, :, mc_ * 128:(mc_ + 1) * 128],
                             pj["t" + st][:, :, o:o + f], True, True)
                    nc.scalar.activation(E[st][:, mc_, :], pt[:], AF.Exp,
                                         bias=expb[:])
                eng = nc.gpsimd if mc_ < S_ON_GPSIMD else nc.vector
                eng.tensor_mul(S[:, mc_, :], E["x"][:, mc_, :],
                               E["y"][:, mc_, :])
                for st in ("x", "y"):
                    w = w_sb[f"wt_g{st}"]
                    pt = psg.tile([128, CI + 1], F32, tag="psg", name="psg")
                    # lhsT = input chunk (c-part, n-cols); rhs = weights
                    # (c-part, ci+colsum free).  wgr is all-zero except the
                    # colsum column (identical per chunk), and the r8
                    # residual only contributes that column: stream 1 col.
                    for kp in range(KC // 2):
                        mmdr(pt[:],
                             s8[st][:, 2 * kp:2 * kp + 2,
                                    mc_ * 128:(mc_ + 1) * 128],
                             w[:, 2 * kp:2 * kp + 2, :CI + 1],
                             kp == 0, False)
                    for kp in range(KC // 2):
                        mmdr(pt[:, CI:CI + 1],
                             r8[st][:, 2 * kp:2 * kp + 2,
                                    mc_ * 128:(mc_ + 1) * 128],
                             wgr[:, :, CI:CI + 1],
                             False, kp == KC // 2 - 1)
                    cast_idx = mc_ * 2 + (0 if st == "x" else 1)
                    if cast_idx < GT_ON_ACT:
                        nc.scalar.copy(gT[st][:, mc_, :], pt[:, :CI])
                    else:
                        nc.vector.tensor_copy(gT[st][:, mc_, :], pt[:, :CI])
                    if st == "x":
                        nc.vector.tensor_scalar_mul(
                            gsx[:, mc_:mc_ + 1], pt[:, CI:CI + 1], c_gs[:])
                    else:
                        nc.vector.scalar_tensor_tensor(
                            gscol[:, mc_:mc_ + 1], pt[:, CI:CI + 1], c_go[:],
                            gsx[:, mc_:mc_ + 1],
                            AluOpType.mult, AluOpType.add)
            return E, S, gT

        def emit_z(s, E):
            """Z row matmuls + zx copy + p1 row; returns p1 (SBUF row)."""
            ptz = {}
            for st in ("x", "y"):
                pt = ps.tile([1, N], F32, tag="ps", name=f"psz{st}")
                ptz[st] = pt
                for j in range(MC // 2):
                    for o, f in NH:
                        mmdr(pt[:, o:o + f], ones8[:, :, 0:1],
                             E[st][:, 2 * j:2 * j + 2, o:o + f],
                             j == 0, j == MC // 2 - 1)
            zx = rtmp.tile([1, N], F32, tag="zx", name="zx", bufs=2)
            nc.scalar.activation(zx[:], ptz["x"][:], AF.Identity,
                                 bias=c_eps[:1, :])
            p1 = rtmp.tile([1, N], F32, tag="p1", name="p1", bufs=2)
            nc.vector.scalar_tensor_tensor(
                p1[:], ptz["y"][:], c_eps[:1, :], zx[:],
                AluOpType.add, AluOpType.mult)
            return p1

        def emit_p3cols(s, p1):
            """transpose p1 row -> [128, MC] cols -> 1/p1^2 (emitted after
            apply so the PE never stalls on the zx/p1 chain)."""
            pcol = psg.tile([128, MC], F32, tag="psg", name="pcol")
            for j in range(MC):
                nc.tensor.transpose(pcol[:, j:j + 1],
                                    p1[:, j * 128:(j + 1) * 128],
                                    ident[:1, :1])
            p2col = rtmp.tile([128, MC], F32, tag="p2col", name="p2col",
                              bufs=2)
            nc.vector.reciprocal(p2col[:], pcol[:])
            p3col = rtmp.tile([128, MC], F32, tag="p3col", name="p3col",
                              bufs=2)
            nc.vector.tensor_mul(p3col[:], p2col[:], p2col[:])
            return p3col

        def emit_apply(s, S, gT):
            """flipped apply: U'(n,ci) psum + stt-reduce -> qcol [128, MC]."""
            qcol = rtmp.tile([128, MC], F32, tag="qcol", name="qcol", bufs=2)
            for j in range(MC):
                ptu = {}
                for st in ("x", "y"):
                    ptu[st] = ps.tile([128, CI], F32, tag="ps",
                                      name=f"psu{st}")
                for mp in range(MC // 2):
                    lhsT = S[:, 2 * mp:2 * mp + 2, j * 128:(j + 1) * 128]
                    for st in ("x", "y"):
                        mmdr(ptu[st], lhsT, gT[st][:, 2 * mp:2 * mp + 2, :],
                             mp == 0, mp == MC // 2 - 1)
                uxb = rtmp.tile([128, CI], BF16, tag="uxb", name="uxb", bufs=2)
                nc.scalar.copy(uxb[:], ptu["x"][:])
                scr = rtmp.tile([128, CI], F32, tag="uscr", name="uscr",
                                bufs=2)
                nc.vector.scalar_tensor_tensor(
                    scr[:], uxb[:], c_inv[:], ptu["y"][:],
                    AluOpType.mult, AluOpType.mult,
                    accum_out=qcol[:, j:j + 1])
            return qcol

        def emit_pooled_head(s, qcol, p3col, gscol):
            qp = rtmp.tile([128, MC], F32, tag="qp", name="qp", bufs=2)
            nc.vector.tensor_mul(qp[:], qcol[:], p3col[:])
            nc.vector.tensor_add(pooledT[:, s, :], qp[:], gscol[:])
            pt = psg.tile([1, HOUT], F32, tag="psg", name="psh")
            for j in range(MC):
                mm(pt[:], pooledT[:, s, j:j + 1], hwT[:, j, :],
                   j == 0, (j == MC - 1) and not has_hb)
            if has_hb:
                mm(pt[:], ones_row[:, :1], hb[:], False, True)
            orow = rows.tile([1, HOUT], F32, tag=f"out_sb{s}",
                             name=f"out_sb{s}")
            nc.scalar.copy(orow[:], pt[:])
            nc.scalar.dma_start(d_out[s:s + 1, :], orow[:])

        # ---------------- software-pipelined emission ----------------
        # PE order per sample: [theta/phi_s (pre-emitted), logits_s, g_s,
        #   theta/phi_{s+1}, Z_s, apply_s, head_s] so the exp_s latency on
        #   ACT hides under g_s + theta/phi_{s+1}.
        pj_next = emit_theta_phi(0)
        pending = None
        for s in range(BPC):
            pj = pj_next
            gsx = rtmp.tile([128, MC], F32, tag="gsx", name="gsx", bufs=2)
            gscol = rtmp.tile([128, MC], F32, tag="gscol", name="gscol",
                              bufs=2)
            if s + 1 < BPC:
                in_tiles.append(load_inputs(s + 1))
            E, S, gT = emit_mid(s, pj, gsx, gscol)
            if pending is not None:
                emit_pooled_head(*pending)
            if s + 1 < BPC:
                pj_next = emit_theta_phi(s + 1)
            p1 = emit_z(s, E)
            qcol = emit_apply(s, S, gT)
            p3col = emit_p3cols(s, p1)
            pending = (s, qcol, p3col, gscol)
        emit_pooled_head(*pending)

    nc.compile()
    return nc


def _prepare(inputs):
    f = lambda k: np.ascontiguousarray(np.asarray(inputs[k], dtype=np.float32))
    bf = lambda a: np.ascontiguousarray(np.asarray(a, dtype=ml_dtypes.bfloat16))
    e4m3 = ml_dtypes.float8_e4m3fn
    sar, opt = f("sar"), f("opt")
    ga = float(np.asarray(inputs["gamma_att"]).reshape(-1)[0])
    go = float(np.asarray(inputs["gamma_opt"]).reshape(-1)[0])
    gs = float(np.asarray(inputs["gamma_sar"]).reshape(-1)[0])
    W_w, W_b = f("W_w"), f("W_b")
    head_w, head_b = f("head_w"), f("head_b")

    wbar = (ga / C) * W_w.sum(axis=0)  # (CI,)
    bbar = (ga / C) * float(W_b.sum())
    hb_eff = head_b + bbar * head_w.sum(axis=1)
    # g biases folded into hb_eff would be wrong (they pass through the
    # attention+product nonlinearity); they are zero in this problem, but
    # keep correctness for small nonzero biases via the wbar-weighted
    # constant shift approximation being exact only at zero.  Assert zero.
    assert not np.any(f("g_sar_b")) and not np.any(f("g_opt_b")), \
        "nonzero g biases unsupported in v2 kernel"
    assert not np.any(f("theta_sar_b")) and not np.any(f("theta_opt_b")) \
        and not np.any(f("phi_sar_b")) and not np.any(f("phi_opt_b")), \
        "nonzero theta/phi biases unsupported in v2 kernel"

    has_hb = bool(np.any(hb_eff))
    global gs_f, go_f
    gs_f, go_f = gs, go

    key = (has_hb, gs, go)
    if key not in _cached:
        _cached[key] = _build(has_hb)
    nc = _cached[key]

    def pack_in(a):
        a = a.reshape(B, KC, 128, N).transpose(0, 2, 1, 3).reshape(B, 128,
                                                                   KC * N)
        return np.ascontiguousarray(a)

    sar_p = pack_in(sar)
    opt_p = pack_in(opt)
    x8 = sar_p.astype(e4m3)
    y8 = opt_p.astype(e4m3)
    rx8 = (sar_p - x8.astype(np.float32)).astype(e4m3)
    ry8 = (opt_p - y8.astype(np.float32)).astype(e4m3)

    GF = 272
    cs_col = np.full((C, 1), CS_W, np.float32)
    gx_w = np.concatenate(
        [GX_SCALE * (f("g_sar_w") * wbar[:, None]).T, cs_col], axis=1)
    gy_w = np.concatenate([GY_SCALE * f("g_opt_w").T, cs_col], axis=1)
    gr_w = np.concatenate(
        [np.zeros((256, CI), np.float32), np.full((256, 1), CS_W, np.float32)],
        axis=1)

    common = {
        "wt_tx": _pack(f("theta_sar_w").T),
        "wt_px": _pack(f("phi_sar_w").T),
        "wt_ty": _pack(f("theta_opt_w").T),
        "wt_py": _pack(f("phi_opt_w").T),
        "wt_gx": _pack(gx_w, pad_to=GF),
        "wt_gy": _pack(gy_w, pad_to=GF),
        "wt_gr": _pack(gr_w, pad_to=GF),
        "hwT": np.ascontiguousarray(
            _pack_bf16(head_w.T)),
        "ones8": np.ones((128, 32), e4m3),
        "ident": np.eye(4, dtype=np.float32),
        "expb": np.full((128, 1), EXP_SHIFT, np.float32),
    }
    if has_hb:
        common["hb"] = bf(hb_eff.reshape(1, HOUT))
        common["ones_row"] = np.ones((1, 128), ml_dtypes.bfloat16)

    in_maps = []
    for c in range(NCORES):
        m = dict(common)
        sl = slice(c * BPC, (c + 1) * BPC)
        m["x8"] = np.ascontiguousarray(x8[sl])
        m["y8"] = np.ascontiguousarray(y8[sl])
        m["rx8"] = np.ascontiguousarray(rx8[sl])
        m["ry8"] = np.ascontiguousarray(ry8[sl])
        in_maps.append(m)
    return nc, in_maps


def _pack_bf16(a):
    a = np.asarray(a, dtype=np.float32)
    r, fdim = a.shape
    k = r // 128
    return np.ascontiguousarray(
        a.reshape(k, 128, fdim).transpose(1, 0, 2).reshape(128, k * fdim)
    ).astype(ml_dtypes.bfloat16)


def kernel(**inputs):
    nc, in_maps = _prepare(inputs)
    res = run_bass_kernel_spmd(nc, in_maps, core_ids=list(range(NCORES)))
    return np.concatenate([res.results[c]["out"] for c in range(NCORES)],
                          axis=0)


if __name__ == "__main__":
    rng = np.random.default_rng(0)
    ins = {
        "sar": rng.standard_normal((B, C, N), dtype=np.float32),
        "opt": rng.standard_normal((B, C, N), dtype=np.float32),
    }
    for nm in ("g_sar", "g_opt", "theta_sar", "theta_opt", "phi_sar",
               "phi_opt"):
        ins[nm + "_w"] = 0.02 * rng.standard_normal((CI, C), dtype=np.float32)
        ins[nm + "_b"] = np.zeros((CI,), np.float32)
    ins["W_w"] = 0.02 * rng.standard_normal((C, CI), dtype=np.float32)
    ins["W_b"] = np.zeros((C,), np.float32)
    ins["head_w"] = 0.02 * rng.standard_normal((HOUT, N), dtype=np.float32)
    ins["head_b"] = np.zeros((HOUT,), np.float32)
    ins["gamma_sar"] = np.asarray([0.3], np.float32)
    ins["gamma_opt"] = np.asarray([1.0], np.float32)
    ins["gamma_att"] = np.asarray([1.0], np.float32)
    out = kernel(**ins)
    print(out.shape, out.dtype, np.abs(out).mean())


# revision 38
# speedup vs baseline: 1.2282x; 1.0121x over previous
"""Trainium2 Bass kernel for nn_CAFF_3100966388292.

Dual-stream (SAR/OPT) cross-attention fusion net, pure data parallel
(4 samples/core on 8 cores). v2: all-fp8 datapath.

Key structure (validated numerically in sim_quant.py; the attention term is
~1e-4 of the output magnitude, so the whole attention path runs in e4m3,
while the dominant residual-colsum path uses an fp8 error-feedback pair
x ~ x8 + r8 which is *more* accurate than a bf16 colsum):
  * inputs only as e4m3 (x8) + e4m3 residual (r8); no bf16 input DMA.
  * theta/phi projections fp8 DoubleRow (as before), outputs pj e4m3.
  * logits computed TRANSPOSED (keys m on partitions) fp8 DR.
  * E = exp(L - 15) stored e4m3; S = (Ex*256)*Ey e4m3 via one
    scalar_tensor_tensor (split across DVE and GpSimd).
  * g projections fp8 DR with wbar = (ga/C)*W_w.sum(0) folded into the
    g_x weights (scaled 2^16; g_y scaled 2^6); the residual colsum rides
    as an extra weight column (2^-9 exact) over both x8 and r8 chunks.
  * softmax denominators Zx, Zy via ones-stationary fp8 DR row matmuls
    over E8; epsilon floors make fp8-dead rows yield 0 attention, not NaN.
  * apply is FLIPPED: lhsT = S chunks (stationary), rhs = gT, so U lands
    as (n-part, ci-free); a scalar_tensor_tensor with accum_out reduces
    (Ux*INV)*Uy over ci directly into a per-n column -> no row-space
    fixup, no 1-lane DVE ops, no transposes of the pooled row.
  * 1/(ZxZy)^2 computed in column space: transpose p1 row once (6 PE
    transposes), then [128,6] reciprocal/square on DVE.
  * per-sample head accumulation + per-sample output DMA -> short tail.
  * DMA spread over 3 hw queues: sync=weights, vector=x-side inputs,
    gpsimd=y-side inputs, scalar=output rows.
"""

import sys
import types

import ml_dtypes
import numpy as np

try:  # pragma: no cover
    import antenv.axon_hooks  # noqa: F401
except ImportError:
    try:
        from trn_agent_boot.trn_boot import _ntff_profile_via_ctypes

        _hook = _ntff_profile_via_ctypes("/opt/axon/libaxon_pjrt.so")
        _mod = types.ModuleType("antenv.axon_hooks")
        _mod.get_axon_ntff_profile_hook = lambda: _hook
        _mod.set_axon_ntff_profile_hook = lambda h: None
        sys.modules["antenv.axon_hooks"] = _mod
    except Exception:
        pass

import concourse.bass as bass  # noqa: F401
import concourse.tile as tile
from concourse import bacc, mybir
from concourse.alu_op_type import AluOpType
from concourse.bass_utils import run_bass_kernel_spmd

F32 = mybir.dt.float32
BF16 = mybir.dt.bfloat16
FP8 = mybir.dt.float8e4

B, C, CI, N, HOUT = 32, 512, 256, 768, 256
NCORES = 8
BPC = B // NCORES
KC = C // 128   # 4 channel chunks
MC = N // 128   # 6 position chunks
CIC = CI // 128  # 2 inner-channel chunks
NH = ((0, 512), (512, 256))  # PSUM-bank-legal free splits of N

EXP_SHIFT = -17.0
GX_SCALE = 2.0 ** 14  # on wbar-folded g_x weights
GY_SCALE = 2.0 ** 6   # on g_y weights
CS_W = 2.0 ** -9      # colsum column weight (exact in e4m3); 1/C = 2^-9
# S = Ex*Ey plain; the e^-15 exp scales cancel exactly through Z in p3
INV_SCALE = 1.0 / (GX_SCALE * GY_SCALE)
Z_EPS = 1e-6

# engine split knobs (tuned from traces)
S_ON_GPSIMD = 2      # how many of the 6 S-chunks go to gpsimd (rest DVE)
GT_ON_ACT = 2        # how many of the 12 gT casts go to ACT (rest DVE)
PJ_ON_ACT = 2        # how many of the 8 pj copies go to ACT (rest DVE)

_cached = {}


def _pack(a, pad_to=None):
    """(R, F) host array -> (128, R//128 * Fp) partition-major e4m3."""
    a = np.asarray(a, dtype=np.float32)
    r, f = a.shape
    if pad_to is not None and f < pad_to:
        a = np.concatenate([a, np.zeros((r, pad_to - f), np.float32)], axis=1)
        f = pad_to
    k = r // 128
    return np.ascontiguousarray(
        a.reshape(k, 128, f).transpose(1, 0, 2).reshape(128, k * f)
    ).astype(ml_dtypes.float8_e4m3fn)


def _build(has_hb):
    nc = bacc.Bacc("TRN2", target_bir_lowering=False, debug=False)
    AF = mybir.ActivationFunctionType
    GF = 272  # padded free width of g-weight chunks (step%16==0 for DR)

    def mm(out, lhsT, rhs, start, stop):
        nc.tensor.matmul(out, lhsT, rhs, start=start, stop=stop)

    def mmdr(out, lhsT, rhs, start, stop):
        nc.tensor.matmul(out, lhsT, rhs, start=start, stop=stop,
                         perf_mode=mybir.MatmulPerfMode.DoubleRow)

    d_x8 = nc.dram_tensor("x8", [BPC, 128, KC * N], FP8, kind="ExternalInput")
    d_y8 = nc.dram_tensor("y8", [BPC, 128, KC * N], FP8, kind="ExternalInput")
    d_rx8 = nc.dram_tensor("rx8", [BPC, 128, KC * N], FP8, kind="ExternalInput")
    d_ry8 = nc.dram_tensor("ry8", [BPC, 128, KC * N], FP8, kind="ExternalInput")
    d_w = {}
    for nm in ("wt_tx", "wt_px", "wt_ty", "wt_py"):
        d_w[nm] = nc.dram_tensor(nm, [128, KC * CI], FP8, kind="ExternalInput")
    for nm in ("wt_gx", "wt_gy"):
        d_w[nm] = nc.dram_tensor(nm, [128, KC * GF], FP8, kind="ExternalInput")
    d_w["wt_gr"] = nc.dram_tensor("wt_gr", [128, 2 * GF], FP8,
                                  kind="ExternalInput")
    d_hwT = nc.dram_tensor("hwT", [128, MC * HOUT], BF16, kind="ExternalInput")
    d_ones8 = nc.dram_tensor("ones8", [128, 32], FP8, kind="ExternalInput")
    d_ident = nc.dram_tensor("ident", [4, 4], F32, kind="ExternalInput")
    d_expb = nc.dram_tensor("expb", [128, 1], F32, kind="ExternalInput")
    if has_hb:
        d_hb = nc.dram_tensor("hb", [1, HOUT], BF16, kind="ExternalInput")
        d_onesr = nc.dram_tensor("ones_row", [1, 128], BF16,
                                 kind="ExternalInput")
    d_out = nc.dram_tensor("out", [BPC, HOUT], F32, kind="ExternalOutput")

    with tile.TileContext(nc) as tc, \
            tc.tile_pool(name="wts", bufs=1) as wts, \
            tc.tile_pool(name="inp", bufs=2) as inp, \
            tc.tile_pool(name="proj", bufs=1) as proj, \
            tc.tile_pool(name="att", bufs=1) as attp, \
            tc.tile_pool(name="rows", bufs=1) as rows, \
            tc.tile_pool(name="rtmp", bufs=4) as rtmp, \
            tc.tile_pool(name="ps", bufs=2, space="PSUM") as ps, \
            tc.tile_pool(name="psg", bufs=2, space="PSUM") as psg:

        # ---- DMA loads: weights on sync q (use-order); x-side inputs on
        # vector q; y-side inputs on gpsimd q ----
        w_sb = {}

        def load_w(nm, cols, eng=nc.sync):
            t = wts.tile([128, KC, cols], FP8, tag=nm, name=nm)
            eng.dma_start(t[:], d_w[nm].ap().rearrange("p (k f) -> p k f", k=KC))
            return t

        # gate the first matmul on as little data as possible
        t = wts.tile([128, KC, CI], FP8, tag="wt_tx", name="wt_tx")
        w_sb["wt_tx"] = t
        nc.sync.dma_start(t[:, 0:2, :], d_w["wt_tx"].ap()[:, :2 * CI]
                          .rearrange("p (k f) -> p k f", k=2))
        x8_0 = inp.tile([128, KC, N], FP8, tag="x8", name="x8")
        nc.scalar.dma_start(x8_0[:, 0:2, :], d_x8[0][:, :2 * N]
                            .rearrange("p (k n) -> p k n", k=2))
        y8_0 = inp.tile([128, KC, N], FP8, tag="y8", name="y8")
        nc.sync.dma_start(y8_0[:, 0:2, :], d_y8[0][:, :2 * N]
                            .rearrange("p (k n) -> p k n", k=2))
        nc.sync.dma_start(t[:, 2:, :], d_w["wt_tx"].ap()[:, 2 * CI:]
                          .rearrange("p (k f) -> p k f", k=KC - 2))
        nc.gpsimd.dma_start(x8_0[:, 2:, :], d_x8[0][:, 2 * N:]
                            .rearrange("p (k n) -> p k n", k=2))
        nc.sync.dma_start(y8_0[:, 2:, :], d_y8[0][:, 2 * N:]
                            .rearrange("p (k n) -> p k n", k=2))
        w_sb["wt_px"] = load_w("wt_px", CI)
        w_sb["wt_ty"] = load_w("wt_ty", CI)
        w_sb["wt_py"] = load_w("wt_py", CI)
        w_sb["wt_gx"] = load_w("wt_gx", GF)
        w_sb["wt_gy"] = load_w("wt_gy", GF)
        wgr = wts.tile([128, 2, GF], FP8, tag="wt_gr", name="wt_gr")
        nc.sync.dma_start(wgr[:], d_w["wt_gr"].ap()
                          .rearrange("p (k f) -> p k f", k=2))
        rx8_0 = inp.tile([128, KC, N], FP8, tag="rx8", name="rx8")
        nc.scalar.dma_start(rx8_0[:], d_rx8[0].rearrange("p (k n) -> p k n",
                                                         k=KC))
        ry8_0 = inp.tile([128, KC, N], FP8, tag="ry8", name="ry8")
        nc.sync.dma_start(ry8_0[:], d_ry8[0].rearrange("p (k n) -> p k n",
                                                         k=KC))
        ones8 = wts.tile([128, 2, 16], FP8, tag="ones8", name="ones8")
        nc.sync.dma_start(ones8[:], d_ones8.ap().rearrange("p (k f) -> p k f",
                                                           k=2))
        ident = wts.tile([4, 4], F32, tag="ident", name="ident")
        nc.sync.dma_start(ident[:], d_ident.ap())
        expb = wts.tile([128, 1], F32, tag="expb", name="expb")
        nc.sync.dma_start(expb[:], d_expb.ap())
        hwT = wts.tile([128, MC, HOUT], BF16, tag="hwT", name="hwT")
        nc.sync.dma_start(hwT[:], d_hwT.ap().rearrange("p (k f) -> p k f",
                                                       k=MC))
        if has_hb:
            hb = wts.tile([1, HOUT], BF16, tag="hb", name="hb")
            nc.sync.dma_start(hb[:], d_hb.ap())
            ones_row = wts.tile([1, 128], BF16, tag="ones_row", name="ones_row")
            nc.sync.dma_start(ones_row[:], d_onesr.ap())

        def load_inputs(s):
            x8 = inp.tile([128, KC, N], FP8, tag="x8", name="x8")
            y8 = inp.tile([128, KC, N], FP8, tag="y8", name="y8")
            rx8 = inp.tile([128, KC, N], FP8, tag="rx8", name="rx8")
            ry8 = inp.tile([128, KC, N], FP8, tag="ry8", name="ry8")
            nc.scalar.dma_start(x8[:], d_x8[s].rearrange("p (k n) -> p k n",
                                                         k=KC))
            nc.sync.dma_start(y8[:], d_y8[s].rearrange("p (k n) -> p k n",
                                                         k=KC))
            nc.scalar.dma_start(rx8[:], d_rx8[s].rearrange("p (k n) -> p k n",
                                                           k=KC))
            nc.sync.dma_start(ry8[:], d_ry8[s].rearrange("p (k n) -> p k n",
                                                           k=KC))
            return x8, y8, rx8, ry8

        in_tiles = [(x8_0, y8_0, rx8_0, ry8_0)]
        pooledT = rows.tile([128, BPC, MC], BF16, tag="pooledT", name="pooledT")

        def const_col(tag, val):
            t = wts.tile([128, 1], F32, tag=tag, name=tag)
            nc.gpsimd.memset(t[:], val)
            return t

        c_eps = const_col("c_eps", Z_EPS)
        c_inv = const_col("c_inv", INV_SCALE)
        c_gs = const_col("c_gs", gs_f)
        c_go = const_col("c_go", go_f)

        # ---------------- per-sample phase emitters ----------------
        # Bank-legal free splits per pair-parity: a [128, 2, 768] f32 psum
        # tile spans 3 banks; the second 768-chunk starts at byte 3072, so
        # its split must be (256, 512) to avoid matmuls crossing banks.
        NHP = (((0, 512), (512, 256)), ((0, 256), (256, 512)))

        def emit_theta_phi(s):
            """theta/phi fp8 DR projections: both cic chunks of a proj go
            into one [128,2,N] psum tile -> ONE wide pj copy per proj."""
            x8, y8 = in_tiles[s][0], in_tiles[s][1]
            s8 = {"x": x8, "y": y8}
            pj = {}
            n_copy = 0
            for st in ("x", "y"):
                for pr in ("t", "p"):
                    w = w_sb[f"wt_{pr}{st}"]
                    dst = proj.tile([128, CIC, N], FP8, tag=f"pj_{pr}{st}",
                                    name=f"pj_{pr}{st}")
                    pj[pr + st] = dst
                    pt = ps.tile([128, 2, N], F32, tag="ps", name="ps")
                    for cic in range(CIC):
                        for kp in range(KC // 2):
                            for o, f in NHP[cic]:
                                mmdr(pt[:, cic, o:o + f],
                                     w[:, 2 * kp:2 * kp + 2,
                                       cic * 128:(cic + 1) * 128],
                                     s8[st][:, 2 * kp:2 * kp + 2, o:o + f],
                                     kp == 0, kp == KC // 2 - 1)
                    if n_copy < PJ_ON_ACT:
                        nc.scalar.copy(dst[:], pt[:])
                    else:
                        nc.vector.tensor_copy(dst[:], pt[:])
                    n_copy += 1
            return pj

        def emit_mid(s, pj, gsx, gscol):
            """Interleaved logits+exp+S and g projections, per mc chunk.

            The logits psums (ps pool) are freed by ACT exp; the g psums
            (psg pool) are freed by the gT casts (ACT/DVE); interleaving
            keeps the PE busy on whichever pool has a free slot instead of
            serially stalling on one consumer.
            """
            x8, y8, rx8, ry8 = in_tiles[s]
            r8 = {"x": rx8, "y": ry8}
            s8 = {"x": x8, "y": y8}
            E = {st: attp.tile([128, MC, N], FP8, tag=f"E{st}", name=f"E{st}")
                 for st in ("x", "y")}
            S = attp.tile([128, MC, N], FP8, tag="S", name="S")
            gT = {st: proj.tile([128, MC, CI], FP8, tag=f"gT{st}",
                                name=f"gT{st}") for st in ("x", "y")}
            for jp in range(MC // 2):
                for st in ("x", "y"):
                    pt = ps.tile([128, 2, N], F32, tag="ps", name="ps")
                    for par in range(2):
                        mc_ = 2 * jp + par
                        for o, f in NHP[par]:
                            mmdr(pt[:, par, o:o + f],
                                 pj["p" + st][:, :, mc_ * 128:(mc_ + 1) * 128],
                                 pj["t" + st][:, :, o:o + f], True, True)
                    nc.scalar.activation(E[st][:, 2 * jp:2 * jp + 2, :],
                                         pt[:], AF.Exp, bias=expb[:])
                eng = nc.gpsimd if jp < S_ON_GPSIMD else nc.vector
                eng.tensor_mul(S[:, 2 * jp:2 * jp + 2, :],
                               E["x"][:, 2 * jp:2 * jp + 2, :],
                               E["y"][:, 2 * jp:2 * jp + 2, :])
                for mc_, st in ((2 * jp, "x"), (2 * jp, "y"),
                                (2 * jp + 1, "x"), (2 * jp + 1, "y")):
                    w = w_sb[f"wt_g{st}"]
                    pt = psg.tile([128, CI + 1], F32, tag="psg", name="psg")
                    # lhsT = input chunk (c-part, n-cols); rhs = weights
                    # (c-part, ci+colsum free).  wgr is all-zero except the
                    # colsum column (identical per chunk), and the r8
                    # residual only contributes that column: stream 1 col.
                    for kp in range(KC // 2):
                        mmdr(pt[:],
                             s8[st][:, 2 * kp:2 * kp + 2,
                                    mc_ * 128:(mc_ + 1) * 128],
                             w[:, 2 * kp:2 * kp + 2, :CI + 1],
                             kp == 0, False)
                    for kp in range(KC // 2):
                        mmdr(pt[:, CI:CI + 1],
                             r8[st][:, 2 * kp:2 * kp + 2,
                                    mc_ * 128:(mc_ + 1) * 128],
                             wgr[:, :, CI:CI + 1],
                             False, kp == KC // 2 - 1)
                    cast_idx = mc_ * 2 + (0 if st == "x" else 1)
                    if cast_idx < GT_ON_ACT:
                        nc.scalar.copy(gT[st][:, mc_, :], pt[:, :CI])
                    else:
                        nc.vector.tensor_copy(gT[st][:, mc_, :], pt[:, :CI])
                    if st == "x":
                        nc.vector.tensor_scalar_mul(
                            gsx[:, mc_:mc_ + 1], pt[:, CI:CI + 1], c_gs[:])
                    else:
                        nc.vector.scalar_tensor_tensor(
                            gscol[:, mc_:mc_ + 1], pt[:, CI:CI + 1], c_go[:],
                            gsx[:, mc_:mc_ + 1],
                            AluOpType.mult, AluOpType.add)
            return E, S, gT

        def emit_z(s, E):
            """Z row matmuls + zx copy + p1 row; returns p1 (SBUF row)."""
            ptz = {}
            for st in ("x", "y"):
                pt = ps.tile([1, N], F32, tag="ps", name=f"psz{st}")
                ptz[st] = pt
                for j in range(MC // 2):
                    for o, f in NH:
                        mmdr(pt[:, o:o + f], ones8[:, :, 0:1],
                             E[st][:, 2 * j:2 * j + 2, o:o + f],
                             j == 0, j == MC // 2 - 1)
            zx = rtmp.tile([1, N], F32, tag="zx", name="zx", bufs=2)
            nc.scalar.activation(zx[:], ptz["x"][:], AF.Identity,
                                 bias=c_eps[:1, :])
            p1 = rtmp.tile([1, N], F32, tag="p1", name="p1", bufs=2)
            nc.vector.scalar_tensor_tensor(
                p1[:], ptz["y"][:], c_eps[:1, :], zx[:],
                AluOpType.add, AluOpType.mult)
            return p1

        def emit_p3cols(s, p1):
            """transpose p1 row -> [128, MC] cols -> 1/p1^2 (emitted after
            apply so the PE never stalls on the zx/p1 chain)."""
            pcol = psg.tile([128, MC], F32, tag="psg", name="pcol")
            for j in range(MC):
                nc.tensor.transpose(pcol[:, j:j + 1],
                                    p1[:, j * 128:(j + 1) * 128],
                                    ident[:1, :1])
            p2col = rtmp.tile([128, MC], F32, tag="p2col", name="p2col",
                              bufs=2)
            nc.vector.reciprocal(p2col[:], pcol[:])
            p3col = rtmp.tile([128, MC], F32, tag="p3col", name="p3col",
                              bufs=2)
            nc.vector.tensor_mul(p3col[:], p2col[:], p2col[:])
            return p3col

        def emit_apply(s, S, gT):
            """flipped apply: U'(n,ci) psum + stt-reduce -> qcol [128, MC]."""
            qcol = rtmp.tile([128, MC], F32, tag="qcol", name="qcol", bufs=2)
            for j in range(MC):
                # U'x and U'y share one 2KB psum bank: [:,0,:] / [:,1,:]
                ptu = psg.tile([128, 2, CI], F32, tag="psg", name="psu")
                for mp in range(MC // 2):
                    lhsT = S[:, 2 * mp:2 * mp + 2, j * 128:(j + 1) * 128]
                    for pi in range(2):
                        mmdr(ptu[:, pi, :], lhsT,
                             gT["x" if pi == 0 else "y"]
                             [:, 2 * mp:2 * mp + 2, :],
                             mp == 0, mp == MC // 2 - 1)
                uxb = rtmp.tile([128, CI], BF16, tag="uxb", name="uxb", bufs=2)
                nc.scalar.copy(uxb[:], ptu[:, 0, :])
                scr = rtmp.tile([128, CI], F32, tag="uscr", name="uscr",
                                bufs=2)
                nc.vector.scalar_tensor_tensor(
                    scr[:], uxb[:], c_inv[:], ptu[:, 1, :],
                    AluOpType.mult, AluOpType.mult,
                    accum_out=qcol[:, j:j + 1])
            return qcol

        def emit_pooled_head(s, qcol, p3col, gscol):
            qp = rtmp.tile([128, MC], F32, tag="qp", name="qp", bufs=2)
            nc.vector.tensor_mul(qp[:], qcol[:], p3col[:])
            nc.vector.tensor_add(pooledT[:, s, :], qp[:], gscol[:])
            pt = psg.tile([1, HOUT], F32, tag="psg", name="psh")
            for j in range(MC):
                mm(pt[:], pooledT[:, s, j:j + 1], hwT[:, j, :],
                   j == 0, (j == MC - 1) and not has_hb)
            if has_hb:
                mm(pt[:], ones_row[:, :1], hb[:], False, True)
            orow = rows.tile([1, HOUT], F32, tag=f"out_sb{s}",
                             name=f"out_sb{s}")
            nc.scalar.copy(orow[:], pt[:])
            nc.scalar.dma_start(d_out[s:s + 1, :], orow[:])

        # ---------------- software-pipelined emission ----------------
        # PE order per sample: [theta/phi_s (pre-emitted), logits_s, g_s,
        #   theta/phi_{s+1}, Z_s, apply_s, head_s] so the exp_s latency on
        #   ACT hides under g_s + theta/phi_{s+1}.
        pj_next = emit_theta_phi(0)
        pending = None
        for s in range(BPC):
            pj = pj_next
            gsx = rtmp.tile([128, MC], F32, tag="gsx", name="gsx", bufs=2)
            gscol = rtmp.tile([128, MC], F32, tag="gscol", name="gscol",
                              bufs=2)
            if s + 1 < BPC:
                in_tiles.append(load_inputs(s + 1))
            E, S, gT = emit_mid(s, pj, gsx, gscol)
            if pending is not None:
                emit_pooled_head(*pending)
            if s + 1 < BPC:
                pj_next = emit_theta_phi(s + 1)
            p1 = emit_z(s, E)
            qcol = emit_apply(s, S, gT)
            p3col = emit_p3cols(s, p1)
            pending = (s, qcol, p3col, gscol)
        emit_pooled_head(*pending)

    nc.compile()
    return nc


def _prepare(inputs):
    f = lambda k: np.ascontiguousarray(np.asarray(inputs[k], dtype=np.float32))
    bf = lambda a: np.ascontiguousarray(np.asarray(a, dtype=ml_dtypes.bfloat16))
    e4m3 = ml_dtypes.float8_e4m3fn
    sar, opt = f("sar"), f("opt")
    ga = float(np.asarray(inputs["gamma_att"]).reshape(-1)[0])
    go = float(np.asarray(inputs["gamma_opt"]).reshape(-1)[0])
    gs = float(np.asarray(inputs["gamma_sar"]).reshape(-1)[0])
    W_w, W_b = f("W_w"), f("W_b")
    head_w, head_b = f("head_w"), f("head_b")

    wbar = (ga / C) * W_w.sum(axis=0)  # (CI,)
    bbar = (ga / C) * float(W_b.sum())
    hb_eff = head_b + bbar * head_w.sum(axis=1)
    # g biases folded into hb_eff would be wrong (they pass through the
    # attention+product nonlinearity); they are zero in this problem, but
    # keep correctness for small nonzero biases via the wbar-weighted
    # constant shift approximation being exact only at zero.  Assert zero.
    assert not np.any(f("g_sar_b")) and not np.any(f("g_opt_b")), \
        "nonzero g biases unsupported in v2 kernel"
    assert not np.any(f("theta_sar_b")) and not np.any(f("theta_opt_b")) \
        and not np.any(f("phi_sar_b")) and not np.any(f("phi_opt_b")), \
        "nonzero theta/phi biases unsupported in v2 kernel"

    has_hb = bool(np.any(hb_eff))
    global gs_f, go_f
    gs_f, go_f = gs, go

    key = (has_hb, gs, go)
    if key not in _cached:
        _cached[key] = _build(has_hb)
    nc = _cached[key]

    def pack_in(a):
        a = a.reshape(B, KC, 128, N).transpose(0, 2, 1, 3).reshape(B, 128,
                                                                   KC * N)
        return np.ascontiguousarray(a)

    sar_p = pack_in(sar)
    opt_p = pack_in(opt)
    x8 = sar_p.astype(e4m3)
    y8 = opt_p.astype(e4m3)
    rx8 = (sar_p - x8.astype(np.float32)).astype(e4m3)
    ry8 = (opt_p - y8.astype(np.float32)).astype(e4m3)

    GF = 272
    cs_col = np.full((C, 1), CS_W, np.float32)
    gx_w = np.concatenate(
        [GX_SCALE * (f("g_sar_w") * wbar[:, None]).T, cs_col], axis=1)
    gy_w = np.concatenate([GY_SCALE * f("g_opt_w").T, cs_col], axis=1)
    gr_w = np.concatenate(
        [np.zeros((256, CI), np.float32), np.full((256, 1), CS_W, np.float32)],
        axis=1)

    common = {
        "wt_tx": _pack(f("theta_sar_w").T),
        "wt_px": _pack(f("phi_sar_w").T),
        "wt_ty": _pack(f("theta_opt_w").T),
        "wt_py": _pack(f("phi_opt_w").T),
        "wt_gx": _pack(gx_w, pad_to=GF),
        "wt_gy": _pack(gy_w, pad_to=GF),
        "wt_gr": _pack(gr_w, pad_to=GF),
        "hwT": np.ascontiguousarray(
            _pack_bf16(head_w.T)),
        "ones8": np.ones((128, 32), e4m3),
        "ident": np.eye(4, dtype=np.float32),
        "expb": np.full((128, 1), EXP_SHIFT, np.float32),
    }
    if has_hb:
        common["hb"] = bf(hb_eff.reshape(1, HOUT))
        common["ones_row"] = np.ones((1, 128), ml_dtypes.bfloat16)

    in_maps = []
    for c in range(NCORES):
        m = dict(common)
        sl = slice(c * BPC, (c + 1) * BPC)
        m["x8"] = np.ascontiguousarray(x8[sl])
        m["y8"] = np.ascontiguousarray(y8[sl])
        m["rx8"] = np.ascontiguousarray(rx8[sl])
        m["ry8"] = np.ascontiguousarray(ry8[sl])
        in_maps.append(m)
    return nc, in_maps


def _pack_bf16(a):
    a = np.asarray(a, dtype=np.float32)
    r, fdim = a.shape
    k = r // 128
    return np.ascontiguousarray(
        a.reshape(k, 128, fdim).transpose(1, 0, 2).reshape(128, k * fdim)
    ).astype(ml_dtypes.bfloat16)


def kernel(**inputs):
    nc, in_maps = _prepare(inputs)
    res = run_bass_kernel_spmd(nc, in_maps, core_ids=list(range(NCORES)))
    return np.concatenate([res.results[c]["out"] for c in range(NCORES)],
                          axis=0)


if __name__ == "__main__":
    rng = np.random.default_rng(0)
    ins = {
        "sar": rng.standard_normal((B, C, N), dtype=np.float32),
        "opt": rng.standard_normal((B, C, N), dtype=np.float32),
    }
    for nm in ("g_sar", "g_opt", "theta_sar", "theta_opt", "phi_sar",
               "phi_opt"):
        ins[nm + "_w"] = 0.02 * rng.standard_normal((CI, C), dtype=np.float32)
        ins[nm + "_b"] = np.zeros((CI,), np.float32)
    ins["W_w"] = 0.02 * rng.standard_normal((C, CI), dtype=np.float32)
    ins["W_b"] = np.zeros((C,), np.float32)
    ins["head_w"] = 0.02 * rng.standard_normal((HOUT, N), dtype=np.float32)
    ins["head_b"] = np.zeros((HOUT,), np.float32)
    ins["gamma_sar"] = np.asarray([0.3], np.float32)
    ins["gamma_opt"] = np.asarray([1.0], np.float32)
    ins["gamma_att"] = np.asarray([1.0], np.float32)
    out = kernel(**ins)
    print(out.shape, out.dtype, np.abs(out).mean())
